# revision 28
# baseline (speedup 1.0000x reference)
"""TRN2 Bass kernel for nn_Base_1348619731207 (gnn_message_passing).

Model:
  graph_out = MLP_graph(mean_pool(x, batch))            # [B, G]
  node_out[b, n] = MLP_node_n(x[b, n, :])               # per-node MLPs, [B, N]
  out = concat([graph_out, node_out], axis=1)           # [B, G + N]

Sharding (8 cores): expert-parallel over the node dim N (64 nodes/core,
per-node head weights sliced with their nodes) + graph-parallel pooling
(16 graphs/core stream their own x rows for the mean-pool + graph head).
Each core reads ~1/8 of every tensor; no collectives.

All streamed data (x, node-head weights, pooling rows) is bf16: halves
HBM traffic vs fp32 (memory-bound regime) and runs the PE at 1 cycle/row
for any moving width, so per-node 128-wide matmuls carry no garbage
columns. Each DMA trigger costs ~0.6us of serial sync-queue time, so all
of a node group's data ships as ONE dma_start with 8 KiB per partition
row (xt | w1 | w2 packed), pool rows as 4 KiB loads, constants as three
packed loads: ~30 triggers total. The graph head (fp32, tiny) is
interleaved into the last node groups so it adds no tail latency.
"""

import numpy as np
import ml_dtypes

import concourse.bass as bass
import concourse.mybir as mybir
from concourse import bacc
from concourse.bass_utils import run_bass_kernel_spmd
from concourse.masks import make_identity
from concourse.tile import TileContext

F32 = mybir.dt.float32
BF16 = mybir.dt.bfloat16
RELU = mybir.ActivationFunctionType.Relu
IDENT = mybir.ActivationFunctionType.Identity
NPBF = ml_dtypes.bfloat16

B, N, H = 128, 512, 256          # graphs, nodes/graph, hidden
DS, D1, D2, G = 128, 256, 128, 32
NCORES = 8
NPC = N // NCORES                # 64 nodes per core
NG = NPC // 4                    # 16 DMA groups of 4 nodes
GPC = B // NCORES                # 16 graphs per core
PT = 68                          # pooling row tiles per core (68*128 = 8704 rows)
PQ = 17                          # pooling DMA loads (4 tiles each)

_CACHE = {}


def _build_nc(zero_bias):
    nc = bacc.Bacc("TRN2", target_bir_lowering=False, debug=False)

    # Per-core inputs (shapes identical on every core).
    # st: per group of 4 nodes, cols = xt[1024] | w1a[1024] | w1b[1024] | w2[1024]
    st_d = nc.dram_tensor("st", [NG, 128, 4096], BF16, kind="ExternalInput")
    # xg: 4 pooling row-tiles per load, cols = t4*256 + h
    xg_d = nc.dram_tensor("xg", [PQ, 128, 1024], BF16, kind="ExternalInput")
    # cA: f32 node-head consts, cols = b1[128] | b2[64] | b3[64]
    cA_d = nc.dram_tensor("cA", [128, 256], F32, kind="ExternalInput")
    # cB: bf16 consts, cols = ind[1088] | w3pad[128]
    cB_d = nc.dram_tensor("cB", [128, 1216], BF16, kind="ExternalInput")
    # cG: f32 graph-head consts,
    # cols = gw1[256] | gw2[128] | gw3[256] | gw4[256] | gw5[32]
    #        | gb1 | gb2 | gb3[2] | gb4 | gb5
    cG_d = nc.dram_tensor("cG", [128, 934], F32, kind="ExternalInput")

    nout_d = nc.dram_tensor("nout", [128, NPC], F32, kind="ExternalOutput")
    gout_d = nc.dram_tensor("gout", [G, GPC], F32, kind="ExternalOutput")

    with TileContext(nc) as tc:
        with (
            tc.tile_pool(name="const", bufs=1) as cst,
            tc.tile_pool(name="stream", bufs=5) as stp,
            tc.tile_pool(name="act", bufs=4) as actp,
            tc.tile_pool(name="xgp", bufs=4) as xgp,
            tc.tile_pool(name="psA", bufs=4, space=bass.MemorySpace.PSUM) as psA,
            tc.tile_pool(name="psB", bufs=2, space=bass.MemorySpace.PSUM) as psB,
            tc.tile_pool(name="psC", bufs=1, space=bass.MemorySpace.PSUM) as psC,
            tc.tile_pool(name="psD", bufs=1, space=bass.MemorySpace.PSUM) as psD,
        ):
            # --- constants (3 packed loads; issued after the first
            # compute-critical stream pieces, see the node loop) ---
            cAt = cst.tile([128, 256], F32)
            cBt = cst.tile([128, 1216], BF16)
            cGt = cst.tile([128, 934], F32)

            b1t = cAt[:, 0:128]          # col = 2n + mh
            b2t = cAt[:, 128:192]        # col = n
            b3t = cAt[:, 192:256]        # col = n (broadcast over partitions)
            indt = cBt[:, 0:1088]        # col = t*16 + graph
            w3t = cBt[:, 1088:1216]      # col = 2n (odd cols zero)

            zeros = cst.tile([128, 256], F32)
            nc.gpsimd.memset(zeros[:], 0.0)

            ident = cst.tile([128, 128], F32)
            make_identity(nc, ident[:])

            def relu_bias(out, in_, bias, use_dve):
                """relu(in_ + bias): ACT or DVE (load balance)."""
                if not use_dve:
                    nc.scalar.activation(out, in_, RELU, bias=bias)
                else:
                    nc.vector.scalar_tensor_tensor(
                        out, in_, bias, zeros[:, :in_.shape[-1]],
                        mybir.AluOpType.add, mybir.AluOpType.max,
                    )

            # node_out staging in SBUF (written 4 nodes at a time)
            nout_sb = cst.tile([128, NPC], F32)
            # pooling accumulator (interleaved with the node loop)
            pp = psD.tile([GPC, 256], F32)

            xg_tiles = {}

            def pool_dma(q):
                xgt = xgp.tile([128, 1024], BF16, tag="xg")
                nc.sync.dma_start(xgt[:], xg_d[q])
                xg_tiles[q] = xgt

            def pool_mms(q):
                xgt = xg_tiles.pop(q)
                for t8 in range(8):
                    t = 8 * q + t8
                    if t >= PT:
                        break
                    nc.tensor.matmul(
                        pp[:],
                        indt[:, t * GPC:(t + 1) * GPC],
                        xgt[:, t8 * 256:(t8 + 1) * 256],
                        start=(t == 0), stop=(t == PT - 1),
                        skip_group_check=True,
                    )

            # --- graph head steps, interleaved into late node groups ---
            gh = {}

            def graph_step(step):
                if step == 0:
                    # relu(mean) then transpose [GPC, 256] -> [256, GPC]
                    xgr = actp.tile([GPC, 256], F32, tag="xgr")
                    nc.scalar.activation(xgr[:], pp[:], RELU)
                    gh["xgr"] = xgr
                elif step == 1:
                    xgt2 = actp.tile([128, 2 * GPC], F32, tag="xgt")
                    for kh in range(2):
                        ptr = psC.tile([128, GPC], F32, tag="p3")
                        nc.tensor.transpose(
                            ptr[:], gh["xgr"][:, kh * 128:(kh + 1) * 128],
                            ident[:GPC, :GPC],
                        )
                        nc.vector.tensor_copy(
                            xgt2[:, kh * GPC:(kh + 1) * GPC], ptr[:]
                        )
                    gh["xgt2"] = xgt2
                elif step == 2:
                    # layer 1: relu(x_graph) @ gs_w1 + gs_b1  (no relu after)
                    g1 = psC.tile([128, GPC], F32, tag="p3")
                    for kh in range(2):
                        nc.tensor.matmul(
                            g1[:], cGt[:, kh * 128:(kh + 1) * 128],
                            gh["xgt2"][:, kh * GPC:(kh + 1) * GPC],
                            start=(kh == 0), stop=(kh == 1),
                        )
                    a1 = actp.tile([128, GPC], F32, tag="ga")
                    nc.scalar.activation(a1[:], g1[:], IDENT,
                                         bias=cGt[:, 928:929])
                    gh["a1"] = a1
                elif step == 3:
                    # layer 2: relu(a1 @ gs_w2 + gs_b2)
                    g2 = psC.tile([128, GPC], F32, tag="p3")
                    nc.tensor.matmul(g2[:], cGt[:, 256:384], gh["a1"][:],
                                     start=True, stop=True)
                    a2 = actp.tile([128, GPC], F32, tag="ga")
                    nc.scalar.activation(a2[:], g2[:], RELU,
                                         bias=cGt[:, 929:930])
                    gh["a2"] = a2
                elif step == 4:
                    # layer 3: relu(a2 @ gh_w1 + gh_b1)  (D1=256 -> two halves)
                    a3 = actp.tile([128, 2 * GPC], F32, tag="ga3")
                    for mh in range(2):
                        g3 = psC.tile([128, GPC], F32, tag="p3")
                        nc.tensor.matmul(
                            g3[:], cGt[:, 384 + mh * 128:384 + (mh + 1) * 128],
                            gh["a2"][:], start=True, stop=True,
                        )
                        nc.scalar.activation(
                            a3[:, mh * GPC:(mh + 1) * GPC], g3[:], RELU,
                            bias=cGt[:, 930 + mh:931 + mh],
                        )
                    gh["a3"] = a3
                elif step == 5:
                    # layer 4: relu(a3 @ gh_w2 + gh_b2)
                    g4 = psC.tile([128, GPC], F32, tag="p3")
                    for kh in range(2):
                        nc.tensor.matmul(
                            g4[:], cGt[:, 640 + kh * 128:640 + (kh + 1) * 128],
                            gh["a3"][:, kh * GPC:(kh + 1) * GPC],
                            start=(kh == 0), stop=(kh == 1),
                        )
                    a4 = actp.tile([128, GPC], F32, tag="ga")
                    nc.scalar.activation(a4[:], g4[:], RELU,
                                         bias=cGt[:, 932:933])
                    gh["a4"] = a4
                elif step == 6:
                    # layer 5: a4 @ gh_w3 + gh_b3
                    g5 = psC.tile([G, GPC], F32, tag="p3")
                    nc.tensor.matmul(g5[:], cGt[:, 896:928], gh["a4"][:],
                                     start=True, stop=True)
                    gout_sb = actp.tile([G, GPC], F32, tag="gout")
                    nc.scalar.activation(gout_sb[:], g5[:], IDENT,
                                         bias=cGt[:G, 933:934])
                    nc.sync.dma_start(gout_d[:], gout_sb[:])

            # --- node loop: flat software pipeline over 64 nodes ---
            # Slot i emits: L1(i); side work (pool / graph-head matmuls,
            # which depend only on prefetched data and act as spacers);
            # L2(i-2); L3(i-4). The 2-slot lag gives each relu ~0.7us to
            # drain before its consumer, so the PE never stalls and stays
            # at its boosted p-state.
            aps = {}
            side = []          # queued pool-matmul spacer thunks
            gh_step = [0]      # next graph-head step to emit
            side_pushed = [0]  # pool loads whose matmuls are already queued

            def emit_l1(i):
                g, v = i // 4, i % 4
                ap = aps[g]
                xt = ap(0, 1024)
                w1o = 1024 + v * 512
                p1 = psA.tile([128, 256], F32, tag="p1")
                for mh in range(2):
                    for kh in range(2):
                        nc.tensor.matmul(
                            p1[:, mh * 128:(mh + 1) * 128],
                            ap(w1o + kh * 256 + mh * 128,
                               w1o + kh * 256 + (mh + 1) * 128),
                            xt[:, v * 256 + kh * 128:v * 256 + (kh + 1) * 128],
                            start=(kh == 0), stop=(kh == 1),
                        )
                h1 = actp.tile([128, 256], BF16, tag="h1", bufs=6)
                for mh in range(2):
                    relu_bias(
                        h1[:, mh * 128:(mh + 1) * 128],
                        p1[:, mh * 128:(mh + 1) * 128],
                        b1t[:, 2 * i + mh:2 * i + mh + 1],
                        use_dve=(mh == 1),
                    )
                return h1

            def emit_l2(i, h1):
                g, v = i // 4, i % 4
                ap = aps[g]
                p2 = psB.tile([128, 128], F32, tag="p2")
                for dh in range(2):
                    nc.tensor.matmul(
                        p2[:],
                        ap(3072 + v * 256 + dh * 128,
                           3072 + v * 256 + (dh + 1) * 128),
                        h1[:, dh * 128:(dh + 1) * 128],
                        start=(dh == 0), stop=(dh == 1),
                    )
                h2 = actp.tile([128, 128], BF16, tag="h2", bufs=6)
                relu_bias(h2, p2[:], b2t[:, i:i + 1], use_dve=(i % 2 == 1))
                return h2

            p3_state = {}

            def emit_l3(i, h2):
                v = i % 4
                if v == 0:
                    p3_state["t"] = psC.tile([128, 8], F32, tag="p3", name="p3")
                p3 = p3_state["t"]
                nc.tensor.matmul(
                    p3[:, 2 * v:2 * v + 2],
                    h2[:],
                    w3t[:, 2 * i:2 * i + 2],
                    start=True, stop=True,
                    skip_group_check=True,
                )
                if v == 3:
                    k = i - 3
                    nc.vector.tensor_add(
                        nout_sb[:, k:k + 4],
                        p3[:, 0:8:2],
                        b3t[:, k:k + 4],
                    )

            def pool_thunk(q, t4):
                t = 4 * q + t4

                def run():
                    xgt = xg_tiles[q]
                    nc.tensor.matmul(
                        pp[:],
                        indt[:, t * GPC:(t + 1) * GPC],
                        xgt[:, t4 * 256:(t4 + 1) * 256],
                        start=(t == 0), stop=(t == PT - 1),
                        skip_group_check=True,
                    )
                return run

            h1s, h2s = {}, {}
            for i in range(NPC):
                g, v = i // 4, i % 4
                if v == 0:
                    if g < 2:
                        pieces = []
                        for pc in range(4):
                            pt = stp.tile([128, 1024], BF16, tag="stP",
                                          bufs=8, name=f"stp{g}{pc}")
                            nc.sync.dma_start(
                                pt[:], st_d[g, :, pc * 1024:(pc + 1) * 1024])
                            pieces.append(pt)
                            if g == 0 and pc == 1:
                                nc.sync.dma_start(cAt[:], cA_d[:])
                                nc.sync.dma_start(cBt[:], cB_d[:])

                        def ap(c0, c1, ps=tuple(pieces)):
                            return ps[c0 // 1024][:, c0 % 1024:
                                                  c0 % 1024 + (c1 - c0)]
                    else:
                        st = stp.tile([128, 4096], BF16, tag="st")
                        nc.sync.dma_start(st[:], st_d[g])

                        def ap(c0, c1, s=st):
                            return s[:, c0:c1]
                    aps[g] = ap
                    if g == 1:
                        nc.sync.dma_start(cGt[:], cG_d[:])
                    # pool loads spread across the stream: 2/group for g<4,
                    # then 1/group until all 17 are out
                    nq = len(xg_tiles)
                    want = min(PQ, 2 * g + 2)
                    for q in range(nq, want):
                        pool_dma(q)
                    # push pool matmuls one group after their DMA so the
                    # in-order PE never waits on an in-flight transfer
                    if g >= 1:
                        prev = min(PQ, 2 * g)
                        for q in range(side_pushed[0], prev):
                            for t4 in range(4):
                                if 4 * q + t4 < PT:
                                    side.append(pool_thunk(q, t4))
                            side_pushed[0] = q + 1

                h1s[i] = emit_l1(i)
                # spacers: up to 2 queued pool ops per slot
                for _ in range(2):
                    if side:
                        side.pop(0)()
                # graph head: one step per half-group once pooling is done
                # (serially-dependent steps spaced ~2 slots apart)
                if not side and i >= 46 and v in (0, 2) and gh_step[0] < 7:
                    graph_step(gh_step[0])
                    gh_step[0] += 1
                if i - 2 >= 0:
                    h2s[i - 2] = emit_l2(i - 2, h1s.pop(i - 2))
                if i - 4 >= 0:
                    emit_l3(i - 4, h2s.pop(i - 4))

            # drain the pipeline
            for s in side:
                s()
            while gh_step[0] < 7:
                graph_step(gh_step[0])
                gh_step[0] += 1
            for i in (NPC - 2, NPC - 1):
                h2s[i] = emit_l2(i, h1s.pop(i))
            for i in range(NPC - 4, NPC):
                emit_l3(i, h2s.pop(i))

            nc.sync.dma_start(nout_d[:], nout_sb[:])

    nc.compile()
    return nc


def _prep_core_inputs(c, xbf, batch, lo_hi, inv_counts, w1bf, nh_b1, w2bf,
                      nh_b2, w3bf, nh_b3, cG):
    ns = slice(c * NPC, (c + 1) * NPC)
    xv = xbf.reshape(B, N, H)

    # xt: [g, p(h%128), v, kh, b] -> [NG, 128, 1024]
    xt = (
        xv[:, ns, :]                              # [b, n, h]
        .reshape(B, NG, 4, 2, 128)                # b, g, v, kh, p
        .transpose(1, 4, 2, 3, 0)                 # g, p, v, kh, b
        .reshape(NG, 128, 1024)
    )

    # w1: per group 2048 cols; col = n2*512 + kh*256 + mh*128 + m
    w1 = (
        w1bf[ns]                                  # [n, h, d1]
        .reshape(NG, 2, 2, 2, 128, 2, 128)        # g, n2, j, kh, p, mh, m
        .transpose(0, 4, 1, 2, 3, 5, 6)           # g, p, n2, j, kh, mh, m
        .reshape(NG, 128, 2048)
    )

    # w2: per group 1024 cols; col = v*256 + dh*128 + e
    w2 = (
        w2bf[ns]                                  # [n, d1, d2]
        .reshape(NG, 4, 2, 128, 128)              # g, v, dh, p, e
        .transpose(0, 3, 1, 2, 4)                 # g, p, v, dh, e
        .reshape(NG, 128, 1024)
    )
    stream = np.ascontiguousarray(np.concatenate([xt, w1, w2], axis=2))

    # w3 padded: col 2n = w3_n, odd cols zero
    w3 = np.zeros((128, 2 * NPC), NPBF)
    w3[:, 0::2] = w3bf[ns, :, 0].T

    cA = np.empty((128, 256), np.float32)
    cA[:, 0:128] = (
        nh_b1[ns].reshape(NPC, 2, 128).transpose(2, 0, 1).reshape(128, 2 * NPC)
    )
    cA[:, 128:192] = nh_b2[ns].T
    cA[:, 192:256] = np.broadcast_to(nh_b3[ns].reshape(1, NPC), (128, NPC))

    # pooling rows for graphs [GPC*c, GPC*(c+1)), 8 row-tiles per DMA load
    lo, hi = lo_hi[c]
    nrows = hi - lo
    xg = np.zeros((PQ * 4 * 128, 256), NPBF)
    xg[:nrows] = xbf[lo:hi]
    xg = np.ascontiguousarray(
        xg.reshape(PQ, 4, 128, 256).transpose(0, 2, 1, 3).reshape(PQ, 128, 1024)
    )
    ind = np.zeros((PT * 128, GPC), np.float32)
    gl = batch[lo:hi] - GPC * c
    ind[np.arange(nrows), gl] = inv_counts[batch[lo:hi]]
    ind = (
        ind.reshape(PT, 128, GPC).transpose(1, 0, 2).reshape(128, PT * GPC)
    ).astype(NPBF)

    cB = np.empty((128, 1216), NPBF)
    cB[:, 0:1088] = ind
    cB[:, 1088:1216] = w3

    return {"st": stream, "xg": xg, "cA": cA, "cB": cB, "cG": cG}


def kernel(x, batch, gs_w1, gs_b1, gs_w2, gs_b2,
           gh_w1, gh_b1, gh_w2, gh_b2, gh_w3, gh_b3,
           nh_w1, nh_b1, nh_w2, nh_b2, nh_w3, nh_b3):
    x = np.asarray(x, np.float32)
    batch = np.asarray(batch, np.int32)

    counts = np.bincount(batch, minlength=B).astype(np.float32)
    inv_counts = np.where(counts > 0, 1.0 / np.maximum(counts, 1), 0.0).astype(
        np.float32
    )
    # row ranges per core (batch is sorted); must fit in the padded tile count
    bounds = np.searchsorted(batch, np.arange(0, B + 1, GPC))
    lo_hi = [(int(bounds[c]), int(bounds[c + 1])) for c in range(NCORES)]
    assert all(hi - lo <= PT * 128 for lo, hi in lo_hi), "graph slice too large"

    xbf = x.astype(NPBF)
    w1bf = np.asarray(nh_w1, np.float32).astype(NPBF)
    w2bf = np.asarray(nh_w2, np.float32).astype(NPBF)
    w3bf = np.asarray(nh_w3, np.float32).astype(NPBF)
    nh_b1 = np.asarray(nh_b1, np.float32)
    nh_b2 = np.asarray(nh_b2, np.float32)
    nh_b3 = np.asarray(nh_b3, np.float32)

    # graph-head consts: one packed f32 tensor (replicated on all cores)
    cG = np.zeros((128, 934), np.float32)
    cG[:, 0:256] = (
        np.asarray(gs_w1, np.float32).reshape(2, 128, 128)
        .transpose(1, 0, 2).reshape(128, 256)
    )
    cG[:, 256:384] = np.asarray(gs_w2, np.float32)
    cG[:, 384:640] = np.asarray(gh_w1, np.float32)
    cG[:, 640:896] = (
        np.asarray(gh_w2, np.float32).reshape(2, 128, 128)
        .transpose(1, 0, 2).reshape(128, 256)
    )
    cG[:, 896:928] = np.asarray(gh_w3, np.float32)
    cG[:, 928] = np.asarray(gs_b1, np.float32)
    cG[:, 929] = np.asarray(gs_b2, np.float32)
    cG[:, 930:932] = np.asarray(gh_b1, np.float32).reshape(2, 128).T
    cG[:, 932] = np.asarray(gh_b2, np.float32)
    cG[:G, 933] = np.asarray(gh_b3, np.float32)

    zero_bias = not (nh_b1.any() or nh_b2.any() or nh_b3.any())
    key = ("nc", zero_bias)
    if key not in _CACHE:
        _CACHE[key] = _build_nc(zero_bias)
    nc = _CACHE[key]

    in_maps = [
        _prep_core_inputs(c, xbf, batch, lo_hi, inv_counts, w1bf, nh_b1,
                          w2bf, nh_b2, w3bf, nh_b3, cG)
        for c in range(NCORES)
    ]

    res = run_bass_kernel_spmd(nc, in_maps, core_ids=list(range(NCORES)))
    _CACHE["last_result"] = res

    out = np.empty((B, G + N), np.float32)
    for c in range(NCORES):
        out[GPC * c:GPC * (c + 1), :G] = res.results[c]["gout"].T
        out[:, G + NPC * c:G + NPC * (c + 1)] = res.results[c]["nout"]
    return out


# revision 29
# speedup vs baseline: 1.0183x; 1.0183x over previous
"""TRN2 Bass kernel for nn_Base_1348619731207 (gnn_message_passing).

Model:
  graph_out = MLP_graph(mean_pool(x, batch))            # [B, G]
  node_out[b, n] = MLP_node_n(x[b, n, :])               # per-node MLPs, [B, N]
  out = concat([graph_out, node_out], axis=1)           # [B, G + N]

Sharding (8 cores): expert-parallel over the node dim N (64 nodes/core,
per-node head weights sliced with their nodes) + graph-parallel pooling
(16 graphs/core stream their own x rows for the mean-pool + graph head).
Each core reads ~1/8 of every tensor; no collectives.

All streamed data (x, node-head weights, pooling rows) is bf16: halves
HBM traffic vs fp32 (memory-bound regime) and runs the PE at 1 cycle/row
for any moving width, so per-node 128-wide matmuls carry no garbage
columns. Each DMA trigger costs ~0.6us of serial sync-queue time, so all
of a node group's data ships as ONE dma_start with 8 KiB per partition
row (xt | w1 | w2 packed), pool rows as 4 KiB loads, constants as three
packed loads: ~30 triggers total. The graph head (fp32, tiny) is
interleaved into the last node groups so it adds no tail latency.
"""

import numpy as np
import ml_dtypes

import concourse.bass as bass
import concourse.mybir as mybir
from concourse import bacc
from concourse.bass_utils import run_bass_kernel_spmd
from concourse.masks import make_identity
from concourse.tile import TileContext

F32 = mybir.dt.float32
BF16 = mybir.dt.bfloat16
F8 = mybir.dt.float8e4
RELU = mybir.ActivationFunctionType.Relu
IDENT = mybir.ActivationFunctionType.Identity
NPBF = ml_dtypes.bfloat16
NPF8 = ml_dtypes.float8_e4m3

B, N, H = 128, 512, 256          # graphs, nodes/graph, hidden
DS, D1, D2, G = 128, 256, 128, 32
NCORES = 8
NPC = N // NCORES                # 64 nodes per core
NG = NPC // 4                    # 16 DMA groups of 4 nodes
GPC = B // NCORES                # 16 graphs per core
PT = 68                          # pooling row tiles per core (68*128 = 8704 rows)
PQ = 17                          # pooling DMA loads (4 tiles each)

_CACHE = {}


def _build_nc(zero_bias):
    nc = bacc.Bacc("TRN2", target_bir_lowering=False, debug=False)

    # Per-core inputs (shapes identical on every core).
    # st: per group of 4 nodes, cols = xt[1024] | w1a[1024] | w1b[1024] | w2[1024]
    st_d = nc.dram_tensor("st", [NG, 128, 4096], BF16, kind="ExternalInput")
    # xg: 4 pooling row-tiles per load, cols = t4*256 + h (fp8: pooling
    # error lands only on the low-magnitude graph branch)
    xg_d = nc.dram_tensor("xg", [PQ, 128, 1024], F8, kind="ExternalInput")
    ind_d = nc.dram_tensor("ind", [128, PT * GPC], F8, kind="ExternalInput")
    # cA: f32 node-head consts, cols = b1[128] | b2[64] | b3[64]
    cA_d = nc.dram_tensor("cA", [128, 256], F32, kind="ExternalInput")
    # cB: bf16 consts, w3pad
    cB_d = nc.dram_tensor("cB", [128, 128], BF16, kind="ExternalInput")
    # cG: f32 graph-head consts,
    # cols = gw1[256] | gw2[128] | gw3[256] | gw4[256] | gw5[32]
    #        | gb1 | gb2 | gb3[2] | gb4 | gb5
    cG_d = nc.dram_tensor("cG", [128, 934], F32, kind="ExternalInput")

    nout_d = nc.dram_tensor("nout", [128, NPC], F32, kind="ExternalOutput")
    gout_d = nc.dram_tensor("gout", [G, GPC], F32, kind="ExternalOutput")

    with TileContext(nc) as tc:
        with (
            tc.tile_pool(name="const", bufs=1) as cst,
            tc.tile_pool(name="stream", bufs=5) as stp,
            tc.tile_pool(name="act", bufs=4) as actp,
            tc.tile_pool(name="xgp", bufs=4) as xgp,
            tc.tile_pool(name="psA", bufs=4, space=bass.MemorySpace.PSUM) as psA,
            tc.tile_pool(name="psB", bufs=2, space=bass.MemorySpace.PSUM) as psB,
            tc.tile_pool(name="psC", bufs=1, space=bass.MemorySpace.PSUM) as psC,
            tc.tile_pool(name="psD", bufs=1, space=bass.MemorySpace.PSUM) as psD,
        ):
            # --- constants (3 packed loads; issued after the first
            # compute-critical stream pieces, see the node loop) ---
            cAt = cst.tile([128, 256], F32)
            cBt = cst.tile([128, 128], BF16)
            indt_t = cst.tile([128, PT * GPC], F8)
            cGt = cst.tile([128, 934], F32)

            b1t = cAt[:, 0:128]          # col = 2n + mh
            b2t = cAt[:, 128:192]        # col = n
            b3t = cAt[:, 192:256]        # col = n (broadcast over partitions)
            indt = indt_t[:]             # col = t*16 + graph
            w3t = cBt[:]                 # col = 2n (odd cols zero)

            zeros = cst.tile([128, 256], F32)
            nc.gpsimd.memset(zeros[:], 0.0)

            ident = cst.tile([128, 128], F32)
            make_identity(nc, ident[:])

            def relu_bias(out, in_, bias, use_dve):
                """relu(in_ + bias): ACT or DVE (load balance)."""
                if not use_dve:
                    nc.scalar.activation(out, in_, RELU, bias=bias)
                else:
                    nc.vector.scalar_tensor_tensor(
                        out, in_, bias, zeros[:, :in_.shape[-1]],
                        mybir.AluOpType.add, mybir.AluOpType.max,
                    )

            # node_out staging in SBUF (written 4 nodes at a time)
            nout_sb = cst.tile([128, NPC], F32)
            # pooling accumulator (interleaved with the node loop)
            pp = psD.tile([GPC, 256], F32)

            xg_tiles = {}

            def pool_dma(q):
                xgt = xgp.tile([128, 1024], F8, tag="xg")
                nc.sync.dma_start(xgt[:], xg_d[q])
                xg_tiles[q] = xgt

            def pool_mms(q):
                xgt = xg_tiles.pop(q)
                for t8 in range(8):
                    t = 8 * q + t8
                    if t >= PT:
                        break
                    nc.tensor.matmul(
                        pp[:],
                        indt[:, t * GPC:(t + 1) * GPC],
                        xgt[:, t8 * 256:(t8 + 1) * 256],
                        start=(t == 0), stop=(t == PT - 1),
                        skip_group_check=True,
                    )

            # --- graph head steps, interleaved into late node groups ---
            gh = {}

            def graph_step(step):
                if step == 0:
                    # relu(mean) then transpose [GPC, 256] -> [256, GPC]
                    xgr = actp.tile([GPC, 256], F32, tag="xgr")
                    nc.scalar.activation(xgr[:], pp[:], RELU)
                    gh["xgr"] = xgr
                elif step == 1:
                    xgt2 = actp.tile([128, 2 * GPC], F32, tag="xgt")
                    for kh in range(2):
                        ptr = psC.tile([128, GPC], F32, tag="p3")
                        nc.tensor.transpose(
                            ptr[:], gh["xgr"][:, kh * 128:(kh + 1) * 128],
                            ident[:GPC, :GPC],
                        )
                        nc.vector.tensor_copy(
                            xgt2[:, kh * GPC:(kh + 1) * GPC], ptr[:]
                        )
                    gh["xgt2"] = xgt2
                elif step == 2:
                    # layer 1: relu(x_graph) @ gs_w1 + gs_b1  (no relu after)
                    g1 = psC.tile([128, GPC], F32, tag="p3")
                    for kh in range(2):
                        nc.tensor.matmul(
                            g1[:], cGt[:, kh * 128:(kh + 1) * 128],
                            gh["xgt2"][:, kh * GPC:(kh + 1) * GPC],
                            start=(kh == 0), stop=(kh == 1),
                        )
                    a1 = actp.tile([128, GPC], F32, tag="ga")
                    nc.scalar.activation(a1[:], g1[:], IDENT,
                                         bias=cGt[:, 928:929])
                    gh["a1"] = a1
                elif step == 3:
                    # layer 2: relu(a1 @ gs_w2 + gs_b2)
                    g2 = psC.tile([128, GPC], F32, tag="p3")
                    nc.tensor.matmul(g2[:], cGt[:, 256:384], gh["a1"][:],
                                     start=True, stop=True)
                    a2 = actp.tile([128, GPC], F32, tag="ga")
                    nc.scalar.activation(a2[:], g2[:], RELU,
                                         bias=cGt[:, 929:930])
                    gh["a2"] = a2
                elif step == 4:
                    # layer 3: relu(a2 @ gh_w1 + gh_b1)  (D1=256 -> two halves)
                    a3 = actp.tile([128, 2 * GPC], F32, tag="ga3")
                    for mh in range(2):
                        g3 = psC.tile([128, GPC], F32, tag="p3")
                        nc.tensor.matmul(
                            g3[:], cGt[:, 384 + mh * 128:384 + (mh + 1) * 128],
                            gh["a2"][:], start=True, stop=True,
                        )
                        nc.scalar.activation(
                            a3[:, mh * GPC:(mh + 1) * GPC], g3[:], RELU,
                            bias=cGt[:, 930 + mh:931 + mh],
                        )
                    gh["a3"] = a3
                elif step == 5:
                    # layer 4: relu(a3 @ gh_w2 + gh_b2)
                    g4 = psC.tile([128, GPC], F32, tag="p3")
                    for kh in range(2):
                        nc.tensor.matmul(
                            g4[:], cGt[:, 640 + kh * 128:640 + (kh + 1) * 128],
                            gh["a3"][:, kh * GPC:(kh + 1) * GPC],
                            start=(kh == 0), stop=(kh == 1),
                        )
                    a4 = actp.tile([128, GPC], F32, tag="ga")
                    nc.scalar.activation(a4[:], g4[:], RELU,
                                         bias=cGt[:, 932:933])
                    gh["a4"] = a4
                elif step == 6:
                    # layer 5: a4 @ gh_w3 + gh_b3
                    g5 = psC.tile([G, GPC], F32, tag="p3")
                    nc.tensor.matmul(g5[:], cGt[:, 896:928], gh["a4"][:],
                                     start=True, stop=True)
                    gout_sb = actp.tile([G, GPC], F32, tag="gout")
                    nc.scalar.activation(gout_sb[:], g5[:], IDENT,
                                         bias=cGt[:G, 933:934])
                    nc.sync.dma_start(gout_d[:], gout_sb[:])

            # --- node loop: flat software pipeline over 64 nodes ---
            # Slot i emits: L1(i); side work (pool / graph-head matmuls,
            # which depend only on prefetched data and act as spacers);
            # L2(i-2); L3(i-4). The 2-slot lag gives each relu ~0.7us to
            # drain before its consumer, so the PE never stalls and stays
            # at its boosted p-state.
            aps = {}
            side = []          # queued pool-matmul spacer thunks
            gh_step = [0]      # next graph-head step to emit
            side_pushed = [0]  # pool loads whose matmuls are already queued

            def emit_l1(i):
                g, v = i // 4, i % 4
                ap = aps[g]
                xt = ap(0, 1024)
                w1o = 1024 + v * 512
                p1 = psA.tile([128, 256], F32, tag="p1")
                for mh in range(2):
                    for kh in range(2):
                        nc.tensor.matmul(
                            p1[:, mh * 128:(mh + 1) * 128],
                            ap(w1o + kh * 256 + mh * 128,
                               w1o + kh * 256 + (mh + 1) * 128),
                            xt[:, v * 256 + kh * 128:v * 256 + (kh + 1) * 128],
                            start=(kh == 0), stop=(kh == 1),
                        )
                h1 = actp.tile([128, 256], BF16, tag="h1", bufs=6)
                for mh in range(2):
                    relu_bias(
                        h1[:, mh * 128:(mh + 1) * 128],
                        p1[:, mh * 128:(mh + 1) * 128],
                        b1t[:, 2 * i + mh:2 * i + mh + 1],
                        use_dve=(mh == 1),
                    )
                return h1

            def emit_l2(i, h1):
                g, v = i // 4, i % 4
                ap = aps[g]
                p2 = psB.tile([128, 128], F32, tag="p2")
                for dh in range(2):
                    nc.tensor.matmul(
                        p2[:],
                        ap(3072 + v * 256 + dh * 128,
                           3072 + v * 256 + (dh + 1) * 128),
                        h1[:, dh * 128:(dh + 1) * 128],
                        start=(dh == 0), stop=(dh == 1),
                    )
                h2 = actp.tile([128, 128], BF16, tag="h2", bufs=6)
                relu_bias(h2, p2[:], b2t[:, i:i + 1], use_dve=(i % 2 == 1))
                return h2

            p3_state = {}

            def emit_l3(i, h2):
                v = i % 4
                if v == 0:
                    p3_state["t"] = psC.tile([128, 8], F32, tag="p3", name="p3")
                p3 = p3_state["t"]
                nc.tensor.matmul(
                    p3[:, 2 * v:2 * v + 2],
                    h2[:],
                    w3t[:, 2 * i:2 * i + 2],
                    start=True, stop=True,
                    skip_group_check=True,
                )
                if v == 3:
                    k = i - 3
                    nc.vector.tensor_add(
                        nout_sb[:, k:k + 4],
                        p3[:, 0:8:2],
                        b3t[:, k:k + 4],
                    )

            def pool_thunk(q, t4):
                t = 4 * q + t4

                def run():
                    xgt = xg_tiles[q]
                    nc.tensor.matmul(
                        pp[:],
                        indt[:, t * GPC:(t + 1) * GPC],
                        xgt[:, t4 * 256:(t4 + 1) * 256],
                        start=(t == 0), stop=(t == PT - 1),
                        skip_group_check=True,
                    )
                return run

            h1s, h2s = {}, {}
            for i in range(NPC):
                g, v = i // 4, i % 4
                if v == 0:
                    if g < 2:
                        pieces = []
                        for pc in range(4):
                            pt = stp.tile([128, 1024], BF16, tag="stP",
                                          bufs=8, name=f"stp{g}{pc}")
                            nc.sync.dma_start(
                                pt[:], st_d[g, :, pc * 1024:(pc + 1) * 1024])
                            pieces.append(pt)
                            if g == 0 and pc == 1:
                                nc.sync.dma_start(cAt[:], cA_d[:])
                                nc.sync.dma_start(cBt[:], cB_d[:])
                                nc.sync.dma_start(indt_t[:], ind_d[:])

                        def ap(c0, c1, ps=tuple(pieces)):
                            return ps[c0 // 1024][:, c0 % 1024:
                                                  c0 % 1024 + (c1 - c0)]
                    else:
                        st = stp.tile([128, 4096], BF16, tag="st")
                        nc.sync.dma_start(st[:], st_d[g])

                        def ap(c0, c1, s=st):
                            return s[:, c0:c1]
                    aps[g] = ap
                    if g == 1:
                        nc.sync.dma_start(cGt[:], cG_d[:])
                    # pool loads spread across the stream: 2/group for g<4,
                    # then 1/group until all 17 are out
                    nq = len(xg_tiles)
                    want = min(PQ, 2 * g + 2)
                    for q in range(nq, want):
                        pool_dma(q)
                    # push pool matmuls one group after their DMA so the
                    # in-order PE never waits on an in-flight transfer
                    if g >= 1:
                        prev = min(PQ, 2 * g)
                        for q in range(side_pushed[0], prev):
                            for t4 in range(4):
                                if 4 * q + t4 < PT:
                                    side.append(pool_thunk(q, t4))
                            side_pushed[0] = q + 1

                h1s[i] = emit_l1(i)
                # spacers: up to 2 queued pool ops per slot
                for _ in range(2):
                    if side:
                        side.pop(0)()
                # graph head: one step per half-group once pooling is done
                # (serially-dependent steps spaced ~2 slots apart)
                if not side and i >= 46 and v in (0, 2) and gh_step[0] < 7:
                    graph_step(gh_step[0])
                    gh_step[0] += 1
                if i - 2 >= 0:
                    h2s[i - 2] = emit_l2(i - 2, h1s.pop(i - 2))
                if i - 4 >= 0:
                    emit_l3(i - 4, h2s.pop(i - 4))

            # drain the pipeline
            for s in side:
                s()
            while gh_step[0] < 7:
                graph_step(gh_step[0])
                gh_step[0] += 1
            for i in (NPC - 2, NPC - 1):
                h2s[i] = emit_l2(i, h1s.pop(i))
            for i in range(NPC - 4, NPC):
                emit_l3(i, h2s.pop(i))

            nc.sync.dma_start(nout_d[:], nout_sb[:])

    nc.compile()
    return nc


def _prep_core_inputs(c, xbf, x8, batch, lo_hi, inv_counts, w1bf, nh_b1,
                      w2bf, nh_b2, w3bf, nh_b3, cG):
    ns = slice(c * NPC, (c + 1) * NPC)
    xv = xbf.reshape(B, N, H)

    # xt: [g, p(h%128), v, kh, b] -> [NG, 128, 1024]
    xt = (
        xv[:, ns, :]                              # [b, n, h]
        .reshape(B, NG, 4, 2, 128)                # b, g, v, kh, p
        .transpose(1, 4, 2, 3, 0)                 # g, p, v, kh, b
        .reshape(NG, 128, 1024)
    )

    # w1: per group 2048 cols; col = n2*512 + kh*256 + mh*128 + m
    w1 = (
        w1bf[ns]                                  # [n, h, d1]
        .reshape(NG, 2, 2, 2, 128, 2, 128)        # g, n2, j, kh, p, mh, m
        .transpose(0, 4, 1, 2, 3, 5, 6)           # g, p, n2, j, kh, mh, m
        .reshape(NG, 128, 2048)
    )

    # w2: per group 1024 cols; col = v*256 + dh*128 + e
    w2 = (
        w2bf[ns]                                  # [n, d1, d2]
        .reshape(NG, 4, 2, 128, 128)              # g, v, dh, p, e
        .transpose(0, 3, 1, 2, 4)                 # g, p, v, dh, e
        .reshape(NG, 128, 1024)
    )
    stream = np.ascontiguousarray(np.concatenate([xt, w1, w2], axis=2))

    # w3 padded: col 2n = w3_n, odd cols zero
    w3 = np.zeros((128, 2 * NPC), NPBF)
    w3[:, 0::2] = w3bf[ns, :, 0].T

    cA = np.empty((128, 256), np.float32)
    cA[:, 0:128] = (
        nh_b1[ns].reshape(NPC, 2, 128).transpose(2, 0, 1).reshape(128, 2 * NPC)
    )
    cA[:, 128:192] = nh_b2[ns].T
    cA[:, 192:256] = np.broadcast_to(nh_b3[ns].reshape(1, NPC), (128, NPC))

    # pooling rows for graphs [GPC*c, GPC*(c+1)), 8 row-tiles per DMA load
    lo, hi = lo_hi[c]
    nrows = hi - lo
    xg = np.zeros((PQ * 4 * 128, 256), NPF8)
    xg[:nrows] = x8[lo:hi]
    xg = np.ascontiguousarray(
        xg.reshape(PQ, 4, 128, 256).transpose(0, 2, 1, 3).reshape(PQ, 128, 1024)
    )
    ind = np.zeros((PT * 128, GPC), np.float32)
    gl = batch[lo:hi] - GPC * c
    ind[np.arange(nrows), gl] = inv_counts[batch[lo:hi]]
    ind = (
        ind.reshape(PT, 128, GPC).transpose(1, 0, 2).reshape(128, PT * GPC)
    ).astype(NPF8)

    return {"st": stream, "xg": xg, "cA": cA, "cB": w3, "ind": ind,
            "cG": cG}


def kernel(x, batch, gs_w1, gs_b1, gs_w2, gs_b2,
           gh_w1, gh_b1, gh_w2, gh_b2, gh_w3, gh_b3,
           nh_w1, nh_b1, nh_w2, nh_b2, nh_w3, nh_b3):
    x = np.asarray(x, np.float32)
    batch = np.asarray(batch, np.int32)

    counts = np.bincount(batch, minlength=B).astype(np.float32)
    inv_counts = np.where(counts > 0, 1.0 / np.maximum(counts, 1), 0.0).astype(
        np.float32
    )
    # row ranges per core (batch is sorted); must fit in the padded tile count
    bounds = np.searchsorted(batch, np.arange(0, B + 1, GPC))
    lo_hi = [(int(bounds[c]), int(bounds[c + 1])) for c in range(NCORES)]
    assert all(hi - lo <= PT * 128 for lo, hi in lo_hi), "graph slice too large"

    xbf = x.astype(NPBF)
    x8 = x.astype(NPF8)
    w1bf = np.asarray(nh_w1, np.float32).astype(NPBF)
    w2bf = np.asarray(nh_w2, np.float32).astype(NPBF)
    w3bf = np.asarray(nh_w3, np.float32).astype(NPBF)
    nh_b1 = np.asarray(nh_b1, np.float32)
    nh_b2 = np.asarray(nh_b2, np.float32)
    nh_b3 = np.asarray(nh_b3, np.float32)

    # graph-head consts: one packed f32 tensor (replicated on all cores)
    cG = np.zeros((128, 934), np.float32)
    cG[:, 0:256] = (
        np.asarray(gs_w1, np.float32).reshape(2, 128, 128)
        .transpose(1, 0, 2).reshape(128, 256)
    )
    cG[:, 256:384] = np.asarray(gs_w2, np.float32)
    cG[:, 384:640] = np.asarray(gh_w1, np.float32)
    cG[:, 640:896] = (
        np.asarray(gh_w2, np.float32).reshape(2, 128, 128)
        .transpose(1, 0, 2).reshape(128, 256)
    )
    cG[:, 896:928] = np.asarray(gh_w3, np.float32)
    cG[:, 928] = np.asarray(gs_b1, np.float32)
    cG[:, 929] = np.asarray(gs_b2, np.float32)
    cG[:, 930:932] = np.asarray(gh_b1, np.float32).reshape(2, 128).T
    cG[:, 932] = np.asarray(gh_b2, np.float32)
    cG[:G, 933] = np.asarray(gh_b3, np.float32)

    zero_bias = not (nh_b1.any() or nh_b2.any() or nh_b3.any())
    key = ("nc", zero_bias)
    if key not in _CACHE:
        _CACHE[key] = _build_nc(zero_bias)
    nc = _CACHE[key]

    in_maps = [
        _prep_core_inputs(c, xbf, x8, batch, lo_hi, inv_counts, w1bf, nh_b1,
                          w2bf, nh_b2, w3bf, nh_b3, cG)
        for c in range(NCORES)
    ]

    res = run_bass_kernel_spmd(nc, in_maps, core_ids=list(range(NCORES)))
    _CACHE["last_result"] = res

    out = np.empty((B, G + N), np.float32)
    for c in range(NCORES):
        out[GPC * c:GPC * (c + 1), :G] = res.results[c]["gout"].T
        out[:, G + NPC * c:G + NPC * (c + 1)] = res.results[c]["nout"]
    return out


# revision 31
# speedup vs baseline: 1.0296x; 1.0110x over previous
"""TRN2 Bass kernel for nn_Base_1348619731207 (gnn_message_passing).

Model:
  graph_out = MLP_graph(mean_pool(x, batch))            # [B, G]
  node_out[b, n] = MLP_node_n(x[b, n, :])               # per-node MLPs, [B, N]
  out = concat([graph_out, node_out], axis=1)           # [B, G + N]

Sharding (8 cores): expert-parallel over the node dim N (64 nodes/core,
per-node head weights sliced with their nodes) + graph-parallel pooling
(16 graphs/core stream their own x rows for the mean-pool + graph head).
Each core reads ~1/8 of every tensor; no collectives.

All streamed data (x, node-head weights, pooling rows) is bf16: halves
HBM traffic vs fp32 (memory-bound regime) and runs the PE at 1 cycle/row
for any moving width, so per-node 128-wide matmuls carry no garbage
columns. Each DMA trigger costs ~0.6us of serial sync-queue time, so all
of a node group's data ships as ONE dma_start with 8 KiB per partition
row (xt | w1 | w2 packed), pool rows as 4 KiB loads, constants as three
packed loads: ~30 triggers total. The graph head (fp32, tiny) is
interleaved into the last node groups so it adds no tail latency.
"""

import numpy as np
import ml_dtypes

import concourse.bass as bass
import concourse.mybir as mybir
from concourse import bacc
from concourse.bass_utils import run_bass_kernel_spmd
from concourse.masks import make_identity
from concourse.tile import TileContext

F32 = mybir.dt.float32
BF16 = mybir.dt.bfloat16
F8 = mybir.dt.float8e4
F8E3 = mybir.dt.float8e3
RELU = mybir.ActivationFunctionType.Relu
IDENT = mybir.ActivationFunctionType.Identity
NPBF = ml_dtypes.bfloat16
NPF8 = ml_dtypes.float8_e4m3
NPF8E3 = ml_dtypes.float8_e3m4

B, N, H = 128, 512, 256          # graphs, nodes/graph, hidden
DS, D1, D2, G = 128, 256, 128, 32
NCORES = 8
NPC = N // NCORES                # 64 nodes per core
NG = NPC // 4                    # 16 DMA groups of 4 nodes
GPC = B // NCORES                # 16 graphs per core
PT = 68                          # pooling row tiles per core (68*128 = 8704 rows)
PQ = 17                          # pooling DMA loads (4 tiles each)

_CACHE = {}


def _build_nc(zero_bias):
    nc = bacc.Bacc("TRN2", target_bir_lowering=False, debug=False)

    # Per-core inputs (shapes identical on every core).
    # st: per group of 4 nodes, cols = xt[1024] | w2[1024]
    st_d = nc.dram_tensor("st", [NG, 128, 2048], BF16, kind="ExternalInput")
    # w1 in fp8 e3m4 (scaled x64 on host; the descale folds into w2 and b1)
    w1_d = nc.dram_tensor("w1", [NG, 128, 2048], F8E3, kind="ExternalInput")
    # xg: 4 pooling row-tiles per load, cols = t4*256 + h (fp8: pooling
    # error lands only on the low-magnitude graph branch)
    xg_d = nc.dram_tensor("xg", [PQ, 128, 1024], F8, kind="ExternalInput")
    ind_d = nc.dram_tensor("ind", [128, PT * GPC], F8, kind="ExternalInput")
    # cA: f32 node-head consts, cols = b1[128] | b2[64] | b3[64]
    cA_d = nc.dram_tensor("cA", [128, 256], F32, kind="ExternalInput")
    # cB: bf16 consts, w3pad
    cB_d = nc.dram_tensor("cB", [128, 128], BF16, kind="ExternalInput")
    # cG: f32 graph-head consts,
    # cols = gw1[256] | gw2[128] | gw3[256] | gw4[256] | gw5[32]
    #        | gb1 | gb2 | gb3[2] | gb4 | gb5
    cG_d = nc.dram_tensor("cG", [128, 934], F32, kind="ExternalInput")

    nout_d = nc.dram_tensor("nout", [128, NPC], F32, kind="ExternalOutput")
    gout_d = nc.dram_tensor("gout", [G, GPC], F32, kind="ExternalOutput")

    with TileContext(nc) as tc:
        with (
            tc.tile_pool(name="const", bufs=1) as cst,
            tc.tile_pool(name="stream", bufs=5) as stp,
            tc.tile_pool(name="act", bufs=4) as actp,
            tc.tile_pool(name="xgp", bufs=4) as xgp,
            tc.tile_pool(name="psA", bufs=4, space=bass.MemorySpace.PSUM) as psA,
            tc.tile_pool(name="psB", bufs=2, space=bass.MemorySpace.PSUM) as psB,
            tc.tile_pool(name="psC", bufs=1, space=bass.MemorySpace.PSUM) as psC,
            tc.tile_pool(name="psD", bufs=1, space=bass.MemorySpace.PSUM) as psD,
        ):
            # --- constants (3 packed loads; issued after the first
            # compute-critical stream pieces, see the node loop) ---
            cAt = cst.tile([128, 256], F32)
            cBt = cst.tile([128, 128], BF16)
            indt_t = cst.tile([128, PT * GPC], F8)
            cGt = cst.tile([128, 934], F32)

            b1t = cAt[:, 0:128]          # col = 2n + mh
            b2t = cAt[:, 128:192]        # col = n
            b3t = cAt[:, 192:256]        # col = n (broadcast over partitions)
            indt = indt_t[:]             # col = t*16 + graph
            w3t = cBt[:]                 # col = 2n (odd cols zero)

            zeros = cst.tile([128, 256], F32)
            nc.gpsimd.memset(zeros[:], 0.0)

            ident = cst.tile([128, 128], F32)
            make_identity(nc, ident[:])

            def relu_bias(out, in_, bias, use_dve):
                """relu(in_ + bias): ACT or DVE (load balance)."""
                if not use_dve:
                    nc.scalar.activation(out, in_, RELU, bias=bias)
                else:
                    nc.vector.scalar_tensor_tensor(
                        out, in_, bias, zeros[:, :in_.shape[-1]],
                        mybir.AluOpType.add, mybir.AluOpType.max,
                    )

            # node_out staging in SBUF (written 4 nodes at a time)
            nout_sb = cst.tile([128, NPC], F32)
            # pooling accumulator (interleaved with the node loop)
            pp = psD.tile([GPC, 256], F32)

            xg_tiles = {}

            def pool_dma(q):
                xgt = xgp.tile([128, 1024], F8, tag="xg")
                nc.sync.dma_start(xgt[:], xg_d[q])
                xg_tiles[q] = xgt

            def pool_mms(q):
                xgt = xg_tiles.pop(q)
                for t8 in range(8):
                    t = 8 * q + t8
                    if t >= PT:
                        break
                    nc.tensor.matmul(
                        pp[:],
                        indt[:, t * GPC:(t + 1) * GPC],
                        xgt[:, t8 * 256:(t8 + 1) * 256],
                        start=(t == 0), stop=(t == PT - 1),
                        skip_group_check=True,
                    )

            # --- graph head steps, interleaved into late node groups ---
            gh = {}

            def graph_step(step):
                if step == 0:
                    # relu(mean) then transpose [GPC, 256] -> [256, GPC]
                    xgr = actp.tile([GPC, 256], F32, tag="xgr")
                    nc.scalar.activation(xgr[:], pp[:], RELU)
                    gh["xgr"] = xgr
                elif step == 1:
                    xgt2 = actp.tile([128, 2 * GPC], F32, tag="xgt")
                    for kh in range(2):
                        ptr = psC.tile([128, GPC], F32, tag="p3")
                        nc.tensor.transpose(
                            ptr[:], gh["xgr"][:, kh * 128:(kh + 1) * 128],
                            ident[:GPC, :GPC],
                        )
                        nc.vector.tensor_copy(
                            xgt2[:, kh * GPC:(kh + 1) * GPC], ptr[:]
                        )
                    gh["xgt2"] = xgt2
                elif step == 2:
                    # layer 1: relu(x_graph) @ gs_w1 + gs_b1  (no relu after)
                    g1 = psC.tile([128, GPC], F32, tag="p3")
                    for kh in range(2):
                        nc.tensor.matmul(
                            g1[:], cGt[:, kh * 128:(kh + 1) * 128],
                            gh["xgt2"][:, kh * GPC:(kh + 1) * GPC],
                            start=(kh == 0), stop=(kh == 1),
                        )
                    a1 = actp.tile([128, GPC], F32, tag="ga")
                    nc.scalar.activation(a1[:], g1[:], IDENT,
                                         bias=cGt[:, 928:929])
                    gh["a1"] = a1
                elif step == 3:
                    # layer 2: relu(a1 @ gs_w2 + gs_b2)
                    g2 = psC.tile([128, GPC], F32, tag="p3")
                    nc.tensor.matmul(g2[:], cGt[:, 256:384], gh["a1"][:],
                                     start=True, stop=True)
                    a2 = actp.tile([128, GPC], F32, tag="ga")
                    nc.scalar.activation(a2[:], g2[:], RELU,
                                         bias=cGt[:, 929:930])
                    gh["a2"] = a2
                elif step == 4:
                    # layer 3: relu(a2 @ gh_w1 + gh_b1)  (D1=256 -> two halves)
                    a3 = actp.tile([128, 2 * GPC], F32, tag="ga3")
                    for mh in range(2):
                        g3 = psC.tile([128, GPC], F32, tag="p3")
                        nc.tensor.matmul(
                            g3[:], cGt[:, 384 + mh * 128:384 + (mh + 1) * 128],
                            gh["a2"][:], start=True, stop=True,
                        )
                        nc.scalar.activation(
                            a3[:, mh * GPC:(mh + 1) * GPC], g3[:], RELU,
                            bias=cGt[:, 930 + mh:931 + mh],
                        )
                    gh["a3"] = a3
                elif step == 5:
                    # layer 4: relu(a3 @ gh_w2 + gh_b2)
                    g4 = psC.tile([128, GPC], F32, tag="p3")
                    for kh in range(2):
                        nc.tensor.matmul(
                            g4[:], cGt[:, 640 + kh * 128:640 + (kh + 1) * 128],
                            gh["a3"][:, kh * GPC:(kh + 1) * GPC],
                            start=(kh == 0), stop=(kh == 1),
                        )
                    a4 = actp.tile([128, GPC], F32, tag="ga")
                    nc.scalar.activation(a4[:], g4[:], RELU,
                                         bias=cGt[:, 932:933])
                    gh["a4"] = a4
                elif step == 6:
                    # layer 5: a4 @ gh_w3 + gh_b3
                    g5 = psC.tile([G, GPC], F32, tag="p3")
                    nc.tensor.matmul(g5[:], cGt[:, 896:928], gh["a4"][:],
                                     start=True, stop=True)
                    gout_sb = actp.tile([G, GPC], F32, tag="gout")
                    nc.scalar.activation(gout_sb[:], g5[:], IDENT,
                                         bias=cGt[:G, 933:934])
                    nc.sync.dma_start(gout_d[:], gout_sb[:])

            # --- node loop: flat software pipeline over 64 nodes ---
            # Slot i emits: L1(i); side work (pool / graph-head matmuls,
            # which depend only on prefetched data and act as spacers);
            # L2(i-2); L3(i-4). The 2-slot lag gives each relu ~0.7us to
            # drain before its consumer, so the PE never stalls and stays
            # at its boosted p-state.
            aps = {}
            side = []          # queued pool-matmul spacer thunks
            gh_step = [0]      # next graph-head step to emit
            side_pushed = [0]  # pool loads whose matmuls are already queued

            def emit_l1(i):
                g, v = i // 4, i % 4
                ap, w1t = aps[g]
                xt = ap(0, 1024)
                w1o = v * 512
                p1 = psA.tile([128, 256], F32, tag="p1")
                for mh in range(2):
                    for kh in range(2):
                        nc.tensor.matmul(
                            p1[:, mh * 128:(mh + 1) * 128],
                            w1t[:, w1o + kh * 256 + mh * 128:
                                 w1o + kh * 256 + (mh + 1) * 128],
                            xt[:, v * 256 + kh * 128:v * 256 + (kh + 1) * 128],
                            start=(kh == 0), stop=(kh == 1),
                        )
                h1 = actp.tile([128, 256], BF16, tag="h1", bufs=6)
                for mh in range(2):
                    relu_bias(
                        h1[:, mh * 128:(mh + 1) * 128],
                        p1[:, mh * 128:(mh + 1) * 128],
                        b1t[:, 2 * i + mh:2 * i + mh + 1],
                        use_dve=(mh == 1),
                    )
                return h1

            def emit_l2(i, h1):
                g, v = i // 4, i % 4
                ap, _ = aps[g]
                p2 = psB.tile([128, 128], F32, tag="p2")
                for dh in range(2):
                    nc.tensor.matmul(
                        p2[:],
                        ap(1024 + v * 256 + dh * 128,
                           1024 + v * 256 + (dh + 1) * 128),
                        h1[:, dh * 128:(dh + 1) * 128],
                        start=(dh == 0), stop=(dh == 1),
                    )
                h2 = actp.tile([128, 128], BF16, tag="h2", bufs=6)
                relu_bias(h2, p2[:], b2t[:, i:i + 1], use_dve=(i % 2 == 1))
                return h2

            p3_state = {}

            def emit_l3(i, h2):
                v = i % 4
                if v == 0:
                    p3_state["t"] = psC.tile([128, 8], F32, tag="p3", name="p3")
                p3 = p3_state["t"]
                nc.tensor.matmul(
                    p3[:, 2 * v:2 * v + 2],
                    h2[:],
                    w3t[:, 2 * i:2 * i + 2],
                    start=True, stop=True,
                    skip_group_check=True,
                )
                if v == 3:
                    k = i - 3
                    nc.vector.tensor_add(
                        nout_sb[:, k:k + 4],
                        p3[:, 0:8:2],
                        b3t[:, k:k + 4],
                    )

            def pool_thunk(q, t4):
                t = 4 * q + t4

                def run():
                    xgt = xg_tiles[q]
                    nc.tensor.matmul(
                        pp[:],
                        indt[:, t * GPC:(t + 1) * GPC],
                        xgt[:, t4 * 256:(t4 + 1) * 256],
                        start=(t == 0), stop=(t == PT - 1),
                        skip_group_check=True,
                    )
                return run

            h1s, h2s = {}, {}
            for i in range(NPC):
                g, v = i // 4, i % 4
                if v == 0:
                    if g < 2:
                        pt0 = stp.tile([128, 1024], BF16, tag="stP",
                                       bufs=4, name=f"stp{g}0")
                        nc.sync.dma_start(pt0[:], st_d[g, :, 0:1024])
                        w1t = stp.tile([128, 2048], F8E3, tag="w1",
                                       name=f"w1t{g}")
                        nc.sync.dma_start(w1t[:], w1_d[g])
                        pt1 = stp.tile([128, 1024], BF16, tag="stP",
                                       bufs=4, name=f"stp{g}1")
                        nc.sync.dma_start(pt1[:], st_d[g, :, 1024:2048])
                        if g == 0:
                            nc.sync.dma_start(cAt[:], cA_d[:])
                            nc.sync.dma_start(cBt[:], cB_d[:])
                            nc.sync.dma_start(indt_t[:], ind_d[:])

                        def ap(c0, c1, ps=(pt0, pt1)):
                            return ps[c0 // 1024][:, c0 % 1024:
                                                  c0 % 1024 + (c1 - c0)]
                    else:
                        st = stp.tile([128, 2048], BF16, tag="st")
                        nc.sync.dma_start(st[:], st_d[g])
                        w1t = stp.tile([128, 2048], F8E3, tag="w1",
                                       name=f"w1t{g}")
                        nc.sync.dma_start(w1t[:], w1_d[g])

                        def ap(c0, c1, s=st):
                            return s[:, c0:c1]
                    aps[g] = (ap, w1t)
                    if g == 1:
                        nc.sync.dma_start(cGt[:], cG_d[:])
                    # pool loads spread across the stream: 2/group for g<4,
                    # then 1/group until all 17 are out
                    nq = len(xg_tiles)
                    want = min(PQ, 2 * g + 2)
                    for q in range(nq, want):
                        pool_dma(q)
                    # push pool matmuls one group after their DMA so the
                    # in-order PE never waits on an in-flight transfer
                    if g >= 1:
                        prev = min(PQ, 2 * g)
                        for q in range(side_pushed[0], prev):
                            for t4 in range(4):
                                if 4 * q + t4 < PT:
                                    side.append(pool_thunk(q, t4))
                            side_pushed[0] = q + 1

                h1s[i] = emit_l1(i)
                # spacers: up to 2 queued pool ops per slot
                for _ in range(2):
                    if side:
                        side.pop(0)()
                # graph head: one step per half-group once pooling is done
                # (serially-dependent steps spaced ~2 slots apart)
                if not side and i >= 46 and v in (0, 2) and gh_step[0] < 7:
                    graph_step(gh_step[0])
                    gh_step[0] += 1
                if i - 2 >= 0:
                    h2s[i - 2] = emit_l2(i - 2, h1s.pop(i - 2))
                if i - 4 >= 0:
                    emit_l3(i - 4, h2s.pop(i - 4))

            # drain the pipeline
            for s in side:
                s()
            while gh_step[0] < 7:
                graph_step(gh_step[0])
                gh_step[0] += 1
            for i in (NPC - 2, NPC - 1):
                h2s[i] = emit_l2(i, h1s.pop(i))
            for i in range(NPC - 4, NPC):
                emit_l3(i, h2s.pop(i))

            nc.sync.dma_start(nout_d[:], nout_sb[:])

    nc.compile()
    return nc


def _prep_core_inputs(c, xbf, x8, batch, lo_hi, inv_counts, w1f8, nh_b1,
                      w2bf, nh_b2, w3bf, nh_b3, cG):
    ns = slice(c * NPC, (c + 1) * NPC)
    xv = xbf.reshape(B, N, H)

    # xt: [g, p(h%128), v, kh, b] -> [NG, 128, 1024]
    xt = (
        xv[:, ns, :]                              # [b, n, h]
        .reshape(B, NG, 4, 2, 128)                # b, g, v, kh, p
        .transpose(1, 4, 2, 3, 0)                 # g, p, v, kh, b
        .reshape(NG, 128, 1024)
    )

    # w1: per group 2048 cols; col = n2*512 + kh*256 + mh*128 + m
    w1 = np.ascontiguousarray(
        w1f8[ns]                                  # [n, h, d1]
        .reshape(NG, 2, 2, 2, 128, 2, 128)        # g, n2, j, kh, p, mh, m
        .transpose(0, 4, 1, 2, 3, 5, 6)           # g, p, n2, j, kh, mh, m
        .reshape(NG, 128, 2048)
    )

    # w2: per group 1024 cols; col = v*256 + dh*128 + e
    w2 = (
        w2bf[ns]                                  # [n, d1, d2]
        .reshape(NG, 4, 2, 128, 128)              # g, v, dh, p, e
        .transpose(0, 3, 1, 2, 4)                 # g, p, v, dh, e
        .reshape(NG, 128, 1024)
    )
    stream = np.ascontiguousarray(np.concatenate([xt, w2], axis=2))

    # w3 padded: col 2n = w3_n, odd cols zero
    w3 = np.zeros((128, 2 * NPC), NPBF)
    w3[:, 0::2] = w3bf[ns, :, 0].T

    cA = np.empty((128, 256), np.float32)
    cA[:, 0:128] = (
        nh_b1[ns].reshape(NPC, 2, 128).transpose(2, 0, 1).reshape(128, 2 * NPC)
    )
    cA[:, 128:192] = nh_b2[ns].T
    cA[:, 192:256] = np.broadcast_to(nh_b3[ns].reshape(1, NPC), (128, NPC))

    # pooling rows for graphs [GPC*c, GPC*(c+1)), 8 row-tiles per DMA load
    lo, hi = lo_hi[c]
    nrows = hi - lo
    xg = np.zeros((PQ * 4 * 128, 256), NPF8)
    xg[:nrows] = x8[lo:hi]
    xg = np.ascontiguousarray(
        xg.reshape(PQ, 4, 128, 256).transpose(0, 2, 1, 3).reshape(PQ, 128, 1024)
    )
    ind = np.zeros((PT * 128, GPC), np.float32)
    gl = batch[lo:hi] - GPC * c
    ind[np.arange(nrows), gl] = inv_counts[batch[lo:hi]]
    ind = (
        ind.reshape(PT, 128, GPC).transpose(1, 0, 2).reshape(128, PT * GPC)
    ).astype(NPF8)

    return {"st": stream, "w1": w1, "xg": xg, "cA": cA, "cB": w3,
            "ind": ind, "cG": cG}


def kernel(x, batch, gs_w1, gs_b1, gs_w2, gs_b2,
           gh_w1, gh_b1, gh_w2, gh_b2, gh_w3, gh_b3,
           nh_w1, nh_b1, nh_w2, nh_b2, nh_w3, nh_b3):
    x = np.asarray(x, np.float32)
    batch = np.asarray(batch, np.int32)

    counts = np.bincount(batch, minlength=B).astype(np.float32)
    inv_counts = np.where(counts > 0, 1.0 / np.maximum(counts, 1), 0.0).astype(
        np.float32
    )
    # row ranges per core (batch is sorted); must fit in the padded tile count
    bounds = np.searchsorted(batch, np.arange(0, B + 1, GPC))
    lo_hi = [(int(bounds[c]), int(bounds[c + 1])) for c in range(NCORES)]
    assert all(hi - lo <= PT * 128 for lo, hi in lo_hi), "graph slice too large"

    xbf = x.astype(NPBF)
    x8 = x.astype(NPF8)
    w1f8 = (np.asarray(nh_w1, np.float32) * 64.0).astype(NPF8E3)
    w2bf = (np.asarray(nh_w2, np.float32) / 64.0).astype(NPBF)
    w3bf = np.asarray(nh_w3, np.float32).astype(NPBF)
    nh_b1 = np.asarray(nh_b1, np.float32) * 64.0
    nh_b2 = np.asarray(nh_b2, np.float32)
    nh_b3 = np.asarray(nh_b3, np.float32)

    # graph-head consts: one packed f32 tensor (replicated on all cores)
    cG = np.zeros((128, 934), np.float32)
    cG[:, 0:256] = (
        np.asarray(gs_w1, np.float32).reshape(2, 128, 128)
        .transpose(1, 0, 2).reshape(128, 256)
    )
    cG[:, 256:384] = np.asarray(gs_w2, np.float32)
    cG[:, 384:640] = np.asarray(gh_w1, np.float32)
    cG[:, 640:896] = (
        np.asarray(gh_w2, np.float32).reshape(2, 128, 128)
        .transpose(1, 0, 2).reshape(128, 256)
    )
    cG[:, 896:928] = np.asarray(gh_w3, np.float32)
    cG[:, 928] = np.asarray(gs_b1, np.float32)
    cG[:, 929] = np.asarray(gs_b2, np.float32)
    cG[:, 930:932] = np.asarray(gh_b1, np.float32).reshape(2, 128).T
    cG[:, 932] = np.asarray(gh_b2, np.float32)
    cG[:G, 933] = np.asarray(gh_b3, np.float32)

    zero_bias = not (nh_b1.any() or nh_b2.any() or nh_b3.any())
    key = ("nc", zero_bias)
    if key not in _CACHE:
        _CACHE[key] = _build_nc(zero_bias)
    nc = _CACHE[key]

    in_maps = [
        _prep_core_inputs(c, xbf, x8, batch, lo_hi, inv_counts, w1f8, nh_b1,
                          w2bf, nh_b2, w3bf, nh_b3, cG)
        for c in range(NCORES)
    ]

    res = run_bass_kernel_spmd(nc, in_maps, core_ids=list(range(NCORES)))
    _CACHE["last_result"] = res

    out = np.empty((B, G + N), np.float32)
    for c in range(NCORES):
        out[GPC * c:GPC * (c + 1), :G] = res.results[c]["gout"].T
        out[:, G + NPC * c:G + NPC * (c + 1)] = res.results[c]["nout"]
    return out


# revision 33
# speedup vs baseline: 1.1512x; 1.1182x over previous
"""TRN2 Bass kernel for nn_Base_1348619731207 (gnn_message_passing).

Model:
  graph_out = MLP_graph(mean_pool(x, batch))            # [B, G]
  node_out[b, n] = MLP_node_n(x[b, n, :])               # per-node MLPs, [B, N]
  out = concat([graph_out, node_out], axis=1)           # [B, G + N]

Sharding (8 cores): expert-parallel over the node dim N (64 nodes/core,
per-node head weights sliced with their nodes) + graph-parallel pooling
(16 graphs/core stream their own x rows for the mean-pool + graph head).
Each core reads ~1/8 of every tensor; no collectives.

Streamed data is quantized to the cheapest dtype the 2e-2 error gate
allows (memory-bound regime; every byte is wall-clock): x and w2 in
bf16, w1 in fp8 e3m4 (scaled x64 on host; the descale folds into w2 and
b1, and the PE accepts mixed e3m4-stationary x bf16-moving operands),
the pooling stream (x rows + inv-count matrix) in fp8 e4m3 (its error
lands only on the low-magnitude graph branch). Per-node 128-wide bf16
matmuls carry no garbage columns (no fp32r small-moving penalty). Each
DMA trigger costs ~0.6us of serial sync-queue time, so loads are packed
to ~2-4 KiB per partition row, ~45 triggers total. The node loop is a
flat software pipeline (L1(i) | pool/graph spacers | L2(i-2) | L3(i-4))
so the PE never waits on a just-issued relu; the graph head (fp32,
tiny) is interleaved into late groups so it adds no tail latency.
"""

import numpy as np
import ml_dtypes

import concourse.bass as bass
import concourse.mybir as mybir
from concourse import bacc
from concourse.bass_utils import run_bass_kernel_spmd
from concourse.masks import make_identity
from concourse.tile import TileContext

F32 = mybir.dt.float32
BF16 = mybir.dt.bfloat16
F8 = mybir.dt.float8e4
F8E3 = mybir.dt.float8e3
RELU = mybir.ActivationFunctionType.Relu
IDENT = mybir.ActivationFunctionType.Identity
NPBF = ml_dtypes.bfloat16
NPF8 = ml_dtypes.float8_e4m3
NPF8E3 = ml_dtypes.float8_e3m4

B, N, H = 128, 512, 256          # graphs, nodes/graph, hidden
DS, D1, D2, G = 128, 256, 128, 32
NCORES = 8
NPC = N // NCORES                # 64 nodes per core
NG = NPC // 4                    # 16 DMA groups of 4 nodes
GPC = B // NCORES                # 16 graphs per core
PT = 68                          # pooling row tiles per core (68*128 = 8704 rows)
PQ = 17                          # pooling DMA loads (4 tiles each)

_CACHE = {}


def _build_nc(zero_bias):
    nc = bacc.Bacc("TRN2", target_bir_lowering=False, debug=False)

    # Per-core inputs (shapes identical on every core).
    # st: per group of 4 nodes, cols = xt[1024] | w2[1024]
    st_d = nc.dram_tensor("st", [NG, 128, 2048], BF16, kind="ExternalInput")
    # w1 in fp8 e3m4 (scaled x64 on host; the descale folds into w2 and b1)
    w1_d = nc.dram_tensor("w1", [NG, 128, 2048], F8E3, kind="ExternalInput")
    # xg: 4 pooling row-tiles per load, cols = t4*256 + h (fp8: pooling
    # error lands only on the low-magnitude graph branch)
    xg_d = nc.dram_tensor("xg", [PQ, 128, 1024], F8, kind="ExternalInput")
    ind_d = nc.dram_tensor("ind", [128, PT * GPC], F8, kind="ExternalInput")
    # cA: f32 node-head consts, cols = b1[128] | b2[64] | b3[64]
    cA_d = nc.dram_tensor("cA", [128, 256], F32, kind="ExternalInput")
    # cB: bf16 consts, w3pad
    cB_d = nc.dram_tensor("cB", [128, 128], BF16, kind="ExternalInput")
    # cG: f32 graph-head consts,
    # cols = gw1[256] | gw2[128] | gw3[256] | gw4[256] | gw5[32]
    #        | gb1 | gb2 | gb3[2] | gb4 | gb5
    cG_d = nc.dram_tensor("cG", [128, 934], F32, kind="ExternalInput")

    nout_d = nc.dram_tensor("nout", [128, NPC], F32, kind="ExternalOutput")
    gout_d = nc.dram_tensor("gout", [G, GPC], F32, kind="ExternalOutput")

    with TileContext(nc) as tc:
        with (
            tc.tile_pool(name="const", bufs=1) as cst,
            tc.tile_pool(name="stream", bufs=5) as stp,
            tc.tile_pool(name="act", bufs=4) as actp,
            tc.tile_pool(name="xgp", bufs=4) as xgp,
            tc.tile_pool(name="psA", bufs=3, space=bass.MemorySpace.PSUM) as psA,
            tc.tile_pool(name="psB", bufs=2, space=bass.MemorySpace.PSUM) as psB,
            tc.tile_pool(name="psC", bufs=1, space=bass.MemorySpace.PSUM) as psC,
            tc.tile_pool(name="psD", bufs=1, space=bass.MemorySpace.PSUM) as psD,
            tc.tile_pool(name="psE", bufs=1, space=bass.MemorySpace.PSUM) as psE,
        ):
            # --- constants (3 packed loads; issued after the first
            # compute-critical stream pieces, see the node loop) ---
            cAt = cst.tile([128, 256], F32)
            cBt = cst.tile([128, 128], BF16)
            indt_t = cst.tile([128, PT * GPC], F8)
            cGt = cst.tile([128, 934], F32)

            b1t = cAt[:, 0:128]          # col = 2n + mh
            b2t = cAt[:, 128:192]        # col = n
            b3t = cAt[:, 192:256]        # col = n (broadcast over partitions)
            indt = indt_t[:]             # col = t*16 + graph
            w3t = cBt[:]                 # col = 2n (odd cols zero)

            zeros = cst.tile([128, 256], F32)
            nc.gpsimd.memset(zeros[:], 0.0)
            zbf = cst.tile([128, 128], BF16)
            nc.gpsimd.memset(zbf[:], 0.0)

            wu_n = [0]

            def filler(n):
                """Dependency-free matmuls that keep the PE streaming (and
                its DVFS p-state boosted) where it would otherwise idle.
                Fresh never-read tile per batch in a dedicated PSUM bank."""
                wu_n[0] += 1
                wu = psE.tile([128, 64], F32, tag="wu",
                              name=f"wu{wu_n[0]}")
                for _ in range(n):
                    nc.tensor.matmul(wu[:], zbf[:], zbf[:, 0:64],
                                     start=True, stop=True,
                                     skip_group_check=True)

            ident = cst.tile([128, 128], F32)
            make_identity(nc, ident[:])

            def relu_bias(out, in_, bias, use_dve):
                """relu(in_ + bias): ACT or DVE (load balance)."""
                if not use_dve:
                    nc.scalar.activation(out, in_, RELU, bias=bias)
                else:
                    nc.vector.scalar_tensor_tensor(
                        out, in_, bias, zeros[:, :in_.shape[-1]],
                        mybir.AluOpType.add, mybir.AluOpType.max,
                    )

            # node_out staging in SBUF (written 4 nodes at a time)
            nout_sb = cst.tile([128, NPC], F32)
            # pooling accumulator (interleaved with the node loop)
            pp = psD.tile([GPC, 256], F32)

            xg_tiles = {}

            def pool_dma(q):
                xgt = xgp.tile([128, 1024], F8, tag="xg")
                nc.sync.dma_start(xgt[:], xg_d[q])
                xg_tiles[q] = xgt

            def pool_mms(q):
                xgt = xg_tiles.pop(q)
                for t8 in range(8):
                    t = 8 * q + t8
                    if t >= PT:
                        break
                    nc.tensor.matmul(
                        pp[:],
                        indt[:, t * GPC:(t + 1) * GPC],
                        xgt[:, t8 * 256:(t8 + 1) * 256],
                        start=(t == 0), stop=(t == PT - 1),
                        skip_group_check=True,
                    )

            # --- graph head steps, interleaved into late node groups ---
            gh = {}

            def graph_step(step):
                if step == 0:
                    # relu(mean) then transpose [GPC, 256] -> [256, GPC]
                    xgr = actp.tile([GPC, 256], F32, tag="xgr")
                    nc.scalar.activation(xgr[:], pp[:], RELU)
                    gh["xgr"] = xgr
                elif step == 1:
                    xgt2 = actp.tile([128, 2 * GPC], F32, tag="xgt")
                    for kh in range(2):
                        ptr = psC.tile([128, GPC], F32, tag="p3")
                        nc.tensor.transpose(
                            ptr[:], gh["xgr"][:, kh * 128:(kh + 1) * 128],
                            ident[:GPC, :GPC],
                        )
                        nc.vector.tensor_copy(
                            xgt2[:, kh * GPC:(kh + 1) * GPC], ptr[:]
                        )
                    gh["xgt2"] = xgt2
                elif step == 2:
                    # layer 1: relu(x_graph) @ gs_w1 + gs_b1  (no relu after)
                    g1 = psC.tile([128, GPC], F32, tag="p3")
                    for kh in range(2):
                        nc.tensor.matmul(
                            g1[:], cGt[:, kh * 128:(kh + 1) * 128],
                            gh["xgt2"][:, kh * GPC:(kh + 1) * GPC],
                            start=(kh == 0), stop=(kh == 1),
                        )
                    a1 = actp.tile([128, GPC], F32, tag="ga")
                    nc.scalar.activation(a1[:], g1[:], IDENT,
                                         bias=cGt[:, 928:929])
                    gh["a1"] = a1
                elif step == 3:
                    # layer 2: relu(a1 @ gs_w2 + gs_b2)
                    g2 = psC.tile([128, GPC], F32, tag="p3")
                    nc.tensor.matmul(g2[:], cGt[:, 256:384], gh["a1"][:],
                                     start=True, stop=True)
                    a2 = actp.tile([128, GPC], F32, tag="ga")
                    nc.scalar.activation(a2[:], g2[:], RELU,
                                         bias=cGt[:, 929:930])
                    gh["a2"] = a2
                elif step == 4:
                    # layer 3: relu(a2 @ gh_w1 + gh_b1)  (D1=256 -> two halves)
                    a3 = actp.tile([128, 2 * GPC], F32, tag="ga3")
                    for mh in range(2):
                        g3 = psC.tile([128, GPC], F32, tag="p3")
                        nc.tensor.matmul(
                            g3[:], cGt[:, 384 + mh * 128:384 + (mh + 1) * 128],
                            gh["a2"][:], start=True, stop=True,
                        )
                        nc.scalar.activation(
                            a3[:, mh * GPC:(mh + 1) * GPC], g3[:], RELU,
                            bias=cGt[:, 930 + mh:931 + mh],
                        )
                    gh["a3"] = a3
                elif step == 5:
                    # layer 4: relu(a3 @ gh_w2 + gh_b2)
                    g4 = psC.tile([128, GPC], F32, tag="p3")
                    for kh in range(2):
                        nc.tensor.matmul(
                            g4[:], cGt[:, 640 + kh * 128:640 + (kh + 1) * 128],
                            gh["a3"][:, kh * GPC:(kh + 1) * GPC],
                            start=(kh == 0), stop=(kh == 1),
                        )
                    a4 = actp.tile([128, GPC], F32, tag="ga")
                    nc.scalar.activation(a4[:], g4[:], RELU,
                                         bias=cGt[:, 932:933])
                    gh["a4"] = a4
                elif step == 6:
                    # layer 5: a4 @ gh_w3 + gh_b3
                    g5 = psC.tile([G, GPC], F32, tag="p3")
                    nc.tensor.matmul(g5[:], cGt[:, 896:928], gh["a4"][:],
                                     start=True, stop=True)
                    gout_sb = actp.tile([G, GPC], F32, tag="gout")
                    nc.scalar.activation(gout_sb[:], g5[:], IDENT,
                                         bias=cGt[:G, 933:934])
                    nc.sync.dma_start(gout_d[:], gout_sb[:])

            # --- node loop: flat software pipeline over 64 nodes ---
            # Slot i emits: L1(i); side work (pool / graph-head matmuls,
            # which depend only on prefetched data and act as spacers);
            # L2(i-2); L3(i-4). The 2-slot lag gives each relu ~0.7us to
            # drain before its consumer, so the PE never stalls and stays
            # at its boosted p-state.
            aps = {}
            side = []          # queued pool-matmul spacer thunks
            gh_step = [0]      # next graph-head step to emit
            side_pushed = [0]  # pool loads whose matmuls are already queued

            def emit_l1(i):
                g, v = i // 4, i % 4
                ap, w1t = aps[g]
                xt = ap(0, 1024)
                w1o = v * 512
                p1 = psA.tile([128, 256], F32, tag="p1")
                for mh in range(2):
                    for kh in range(2):
                        nc.tensor.matmul(
                            p1[:, mh * 128:(mh + 1) * 128],
                            w1t[:, w1o + kh * 256 + mh * 128:
                                 w1o + kh * 256 + (mh + 1) * 128],
                            xt[:, v * 256 + kh * 128:v * 256 + (kh + 1) * 128],
                            start=(kh == 0), stop=(kh == 1),
                        )
                h1 = actp.tile([128, 256], BF16, tag="h1", bufs=6)
                for mh in range(2):
                    relu_bias(
                        h1[:, mh * 128:(mh + 1) * 128],
                        p1[:, mh * 128:(mh + 1) * 128],
                        b1t[:, 2 * i + mh:2 * i + mh + 1],
                        use_dve=(mh == 1),
                    )
                return h1

            def emit_l2(i, h1):
                g, v = i // 4, i % 4
                ap, _ = aps[g]
                p2 = psB.tile([128, 128], F32, tag="p2")
                for dh in range(2):
                    nc.tensor.matmul(
                        p2[:],
                        ap(1024 + v * 256 + dh * 128,
                           1024 + v * 256 + (dh + 1) * 128),
                        h1[:, dh * 128:(dh + 1) * 128],
                        start=(dh == 0), stop=(dh == 1),
                    )
                h2 = actp.tile([128, 128], BF16, tag="h2", bufs=6)
                relu_bias(h2, p2[:], b2t[:, i:i + 1], use_dve=(i % 2 == 1))
                return h2

            p3_state = {}

            def emit_l3(i, h2):
                v = i % 4
                if v == 0:
                    p3_state["t"] = psC.tile([128, 8], F32, tag="p3", name="p3")
                p3 = p3_state["t"]
                nc.tensor.matmul(
                    p3[:, 2 * v:2 * v + 2],
                    h2[:],
                    w3t[:, 2 * i:2 * i + 2],
                    start=True, stop=True,
                    skip_group_check=True,
                )
                if v == 3:
                    k = i - 3
                    nc.vector.tensor_add(
                        nout_sb[:, k:k + 4],
                        p3[:, 0:8:2],
                        b3t[:, k:k + 4],
                    )

            def pool_thunk(q, t4):
                t = 4 * q + t4

                def run():
                    xgt = xg_tiles[q]
                    nc.tensor.matmul(
                        pp[:],
                        indt[:, t * GPC:(t + 1) * GPC],
                        xgt[:, t4 * 256:(t4 + 1) * 256],
                        start=(t == 0), stop=(t == PT - 1),
                        skip_group_check=True,
                    )
                return run

            filler(40)
            h1s, h2s = {}, {}
            for i in range(NPC):
                g, v = i // 4, i % 4
                if v == 0:
                    if g < 2:
                        pt0 = stp.tile([128, 1024], BF16, tag="stP",
                                       bufs=4, name=f"stp{g}0")
                        nc.sync.dma_start(pt0[:], st_d[g, :, 0:1024])
                        w1t = stp.tile([128, 2048], F8E3, tag="w1",
                                       name=f"w1t{g}")
                        nc.sync.dma_start(w1t[:], w1_d[g])
                        pt1 = stp.tile([128, 1024], BF16, tag="stP",
                                       bufs=4, name=f"stp{g}1")
                        nc.sync.dma_start(pt1[:], st_d[g, :, 1024:2048])
                        if g == 0:
                            nc.sync.dma_start(cAt[:], cA_d[:])
                            nc.sync.dma_start(cBt[:], cB_d[:])
                            nc.sync.dma_start(indt_t[:], ind_d[:])

                        def ap(c0, c1, ps=(pt0, pt1)):
                            return ps[c0 // 1024][:, c0 % 1024:
                                                  c0 % 1024 + (c1 - c0)]
                    else:
                        st = stp.tile([128, 2048], BF16, tag="st")
                        nc.sync.dma_start(st[:], st_d[g])
                        w1t = stp.tile([128, 2048], F8E3, tag="w1",
                                       name=f"w1t{g}")
                        nc.sync.dma_start(w1t[:], w1_d[g])

                        def ap(c0, c1, s=st):
                            return s[:, c0:c1]
                    aps[g] = (ap, w1t)
                    if g == 1:
                        nc.sync.dma_start(cGt[:], cG_d[:])
                    # pool loads spread across the stream: 2/group for g<4,
                    # then 1/group until all 17 are out
                    nq = len(xg_tiles)
                    want = min(PQ, 2 * g + 2)
                    for q in range(nq, want):
                        pool_dma(q)
                    # push pool matmuls one group after their DMA so the
                    # in-order PE never waits on an in-flight transfer
                    if g >= 1:
                        prev = min(PQ, 2 * g)
                        for q in range(side_pushed[0], prev):
                            for t4 in range(4):
                                if 4 * q + t4 < PT:
                                    side.append(pool_thunk(q, t4))
                            side_pushed[0] = q + 1

                if v == 0 and 1 <= g <= 12:
                    filler(4)
                h1s[i] = emit_l1(i)
                # spacers: up to 2 queued pool ops per slot
                for _ in range(2):
                    if side:
                        side.pop(0)()
                # graph head: one step per half-group once pooling is done
                # (serially-dependent steps spaced ~2 slots apart)
                if not side and i >= 46 and v in (0, 2) and gh_step[0] < 7:
                    graph_step(gh_step[0])
                    gh_step[0] += 1
                if i - 2 >= 0:
                    h2s[i - 2] = emit_l2(i - 2, h1s.pop(i - 2))
                if i - 4 >= 0:
                    emit_l3(i - 4, h2s.pop(i - 4))

            # drain the pipeline
            for s in side:
                s()
            while gh_step[0] < 7:
                graph_step(gh_step[0])
                gh_step[0] += 1
            for i in (NPC - 2, NPC - 1):
                h2s[i] = emit_l2(i, h1s.pop(i))
            for i in range(NPC - 4, NPC):
                emit_l3(i, h2s.pop(i))

            nc.sync.dma_start(nout_d[:], nout_sb[:])

    nc.compile()
    return nc


def _prep_core_inputs(c, xbf, x8, batch, lo_hi, inv_counts, w1f8, nh_b1,
                      w2bf, nh_b2, w3bf, nh_b3, cG):
    ns = slice(c * NPC, (c + 1) * NPC)
    xv = xbf.reshape(B, N, H)

    # xt: [g, p(h%128), v, kh, b] -> [NG, 128, 1024]
    xt = (
        xv[:, ns, :]                              # [b, n, h]
        .reshape(B, NG, 4, 2, 128)                # b, g, v, kh, p
        .transpose(1, 4, 2, 3, 0)                 # g, p, v, kh, b
        .reshape(NG, 128, 1024)
    )

    # w1: per group 2048 cols; col = n2*512 + kh*256 + mh*128 + m
    w1 = np.ascontiguousarray(
        w1f8[ns]                                  # [n, h, d1]
        .reshape(NG, 2, 2, 2, 128, 2, 128)        # g, n2, j, kh, p, mh, m
        .transpose(0, 4, 1, 2, 3, 5, 6)           # g, p, n2, j, kh, mh, m
        .reshape(NG, 128, 2048)
    )

    # w2: per group 1024 cols; col = v*256 + dh*128 + e
    w2 = (
        w2bf[ns]                                  # [n, d1, d2]
        .reshape(NG, 4, 2, 128, 128)              # g, v, dh, p, e
        .transpose(0, 3, 1, 2, 4)                 # g, p, v, dh, e
        .reshape(NG, 128, 1024)
    )
    stream = np.ascontiguousarray(np.concatenate([xt, w2], axis=2))

    # w3 padded: col 2n = w3_n, odd cols zero
    w3 = np.zeros((128, 2 * NPC), NPBF)
    w3[:, 0::2] = w3bf[ns, :, 0].T

    cA = np.empty((128, 256), np.float32)
    cA[:, 0:128] = (
        nh_b1[ns].reshape(NPC, 2, 128).transpose(2, 0, 1).reshape(128, 2 * NPC)
    )
    cA[:, 128:192] = nh_b2[ns].T
    cA[:, 192:256] = np.broadcast_to(nh_b3[ns].reshape(1, NPC), (128, NPC))

    # pooling rows for graphs [GPC*c, GPC*(c+1)), 8 row-tiles per DMA load
    lo, hi = lo_hi[c]
    nrows = hi - lo
    xg = np.zeros((PQ * 4 * 128, 256), NPF8)
    xg[:nrows] = x8[lo:hi]
    xg = np.ascontiguousarray(
        xg.reshape(PQ, 4, 128, 256).transpose(0, 2, 1, 3).reshape(PQ, 128, 1024)
    )
    ind = np.zeros((PT * 128, GPC), np.float32)
    gl = batch[lo:hi] - GPC * c
    ind[np.arange(nrows), gl] = inv_counts[batch[lo:hi]]
    ind = (
        ind.reshape(PT, 128, GPC).transpose(1, 0, 2).reshape(128, PT * GPC)
    ).astype(NPF8)

    return {"st": stream, "w1": w1, "xg": xg, "cA": cA, "cB": w3,
            "ind": ind, "cG": cG}


def kernel(x, batch, gs_w1, gs_b1, gs_w2, gs_b2,
           gh_w1, gh_b1, gh_w2, gh_b2, gh_w3, gh_b3,
           nh_w1, nh_b1, nh_w2, nh_b2, nh_w3, nh_b3):
    x = np.asarray(x, np.float32)
    batch = np.asarray(batch, np.int32)

    counts = np.bincount(batch, minlength=B).astype(np.float32)
    inv_counts = np.where(counts > 0, 1.0 / np.maximum(counts, 1), 0.0).astype(
        np.float32
    )
    # row ranges per core (batch is sorted); must fit in the padded tile count
    bounds = np.searchsorted(batch, np.arange(0, B + 1, GPC))
    lo_hi = [(int(bounds[c]), int(bounds[c + 1])) for c in range(NCORES)]
    assert all(hi - lo <= PT * 128 for lo, hi in lo_hi), "graph slice too large"

    xbf = x.astype(NPBF)
    x8 = x.astype(NPF8)
    w1f8 = (np.asarray(nh_w1, np.float32) * 64.0).astype(NPF8E3)
    w2bf = (np.asarray(nh_w2, np.float32) / 64.0).astype(NPBF)
    w3bf = np.asarray(nh_w3, np.float32).astype(NPBF)
    nh_b1 = np.asarray(nh_b1, np.float32) * 64.0
    nh_b2 = np.asarray(nh_b2, np.float32)
    nh_b3 = np.asarray(nh_b3, np.float32)

    # graph-head consts: one packed f32 tensor (replicated on all cores)
    cG = np.zeros((128, 934), np.float32)
    cG[:, 0:256] = (
        np.asarray(gs_w1, np.float32).reshape(2, 128, 128)
        .transpose(1, 0, 2).reshape(128, 256)
    )
    cG[:, 256:384] = np.asarray(gs_w2, np.float32)
    cG[:, 384:640] = np.asarray(gh_w1, np.float32)
    cG[:, 640:896] = (
        np.asarray(gh_w2, np.float32).reshape(2, 128, 128)
        .transpose(1, 0, 2).reshape(128, 256)
    )
    cG[:, 896:928] = np.asarray(gh_w3, np.float32)
    cG[:, 928] = np.asarray(gs_b1, np.float32)
    cG[:, 929] = np.asarray(gs_b2, np.float32)
    cG[:, 930:932] = np.asarray(gh_b1, np.float32).reshape(2, 128).T
    cG[:, 932] = np.asarray(gh_b2, np.float32)
    cG[:G, 933] = np.asarray(gh_b3, np.float32)

    zero_bias = not (nh_b1.any() or nh_b2.any() or nh_b3.any())
    key = ("nc", zero_bias)
    if key not in _CACHE:
        _CACHE[key] = _build_nc(zero_bias)
    nc = _CACHE[key]

    in_maps = [
        _prep_core_inputs(c, xbf, x8, batch, lo_hi, inv_counts, w1f8, nh_b1,
                          w2bf, nh_b2, w3bf, nh_b3, cG)
        for c in range(NCORES)
    ]

    res = run_bass_kernel_spmd(nc, in_maps, core_ids=list(range(NCORES)))
    _CACHE["last_result"] = res

    out = np.empty((B, G + N), np.float32)
    for c in range(NCORES):
        out[GPC * c:GPC * (c + 1), :G] = res.results[c]["gout"].T
        out[:, G + NPC * c:G + NPC * (c + 1)] = res.results[c]["nout"]
    return out


# revision 34
# speedup vs baseline: 1.2000x; 1.0424x over previous
"""TRN2 Bass kernel for nn_Base_1348619731207 (gnn_message_passing).

Model:
  graph_out = MLP_graph(mean_pool(x, batch))            # [B, G]
  node_out[b, n] = MLP_node_n(x[b, n, :])               # per-node MLPs, [B, N]
  out = concat([graph_out, node_out], axis=1)           # [B, G + N]

Sharding (8 cores): expert-parallel over the node dim N (64 nodes/core,
per-node head weights sliced with their nodes) + graph-parallel pooling
(16 graphs/core stream their own x rows for the mean-pool + graph head).
Each core reads ~1/8 of every tensor; no collectives.

Streamed data is quantized to the cheapest dtype the 2e-2 error gate
allows (memory-bound regime; every byte is wall-clock): x and w2 in
bf16, w1 in fp8 e3m4 (scaled x64 on host; the descale folds into w2 and
b1, and the PE accepts mixed e3m4-stationary x bf16-moving operands),
the pooling stream (x rows + inv-count matrix) in fp8 e4m3 (its error
lands only on the low-magnitude graph branch). Per-node 128-wide bf16
matmuls carry no garbage columns (no fp32r small-moving penalty). Each
DMA trigger costs ~0.6us of serial sync-queue time, so loads are packed
to ~2-4 KiB per partition row, ~45 triggers total. The node loop is a
flat software pipeline (L1(i) | pool/graph spacers | L2(i-2) | L3(i-4))
so the PE never waits on a just-issued relu; the graph head (fp32,
tiny) is interleaved into late groups so it adds no tail latency.
"""

import numpy as np
import ml_dtypes

import concourse.bass as bass
import concourse.mybir as mybir
from concourse import bacc
from concourse.bass_utils import run_bass_kernel_spmd
from concourse.masks import make_identity
from concourse.tile import TileContext

F32 = mybir.dt.float32
BF16 = mybir.dt.bfloat16
F8 = mybir.dt.float8e4
F8E3 = mybir.dt.float8e3
RELU = mybir.ActivationFunctionType.Relu
IDENT = mybir.ActivationFunctionType.Identity
NPBF = ml_dtypes.bfloat16
NPF8 = ml_dtypes.float8_e4m3
NPF8E3 = ml_dtypes.float8_e3m4

B, N, H = 128, 512, 256          # graphs, nodes/graph, hidden
DS, D1, D2, G = 128, 256, 128, 32
NCORES = 8
NPC = N // NCORES                # 64 nodes per core
NG = NPC // 4                    # 16 DMA groups of 4 nodes
GPC = B // NCORES                # 16 graphs per core
PT = 68                          # pooling row tiles per core (68*128 = 8704 rows)
PQ = 17                          # pooling DMA loads (4 tiles each)

_CACHE = {}


def _build_nc(zero_bias):
    nc = bacc.Bacc("TRN2", target_bir_lowering=False, debug=False)

    # Per-core inputs (shapes identical on every core).
    # st: per group of 4 nodes, cols = xt[1024] | w2[1024]
    st_d = nc.dram_tensor("st", [NG, 128, 2048], BF16, kind="ExternalInput")
    # w1 in fp8 e3m4 (scaled x64 on host; the descale folds into w2 and b1)
    w1_d = nc.dram_tensor("w1", [NG, 128, 2048], F8E3, kind="ExternalInput")
    # xg: 4 pooling row-tiles per load, cols = t4*256 + h (fp8: pooling
    # error lands only on the low-magnitude graph branch)
    xg_d = nc.dram_tensor("xg", [PQ, 128, 1024], F8, kind="ExternalInput")
    ind_d = nc.dram_tensor("ind", [128, PT * GPC], F8, kind="ExternalInput")
    # cA: f32 node-head consts, cols = b1[128] | b2[64] | b3[64]
    cA_d = nc.dram_tensor("cA", [128, 256], F32, kind="ExternalInput")
    # cB: bf16 consts, w3pad
    cB_d = nc.dram_tensor("cB", [128, 128], BF16, kind="ExternalInput")
    # cG: f32 graph-head consts,
    # cols = gw1[256] | gw2[128] | gw3[256] | gw4[256] | gw5[32]
    #        | gb1 | gb2 | gb3[2] | gb4 | gb5
    cG_d = nc.dram_tensor("cG", [128, 934], F32, kind="ExternalInput")

    nout_d = nc.dram_tensor("nout", [128, NPC], F32, kind="ExternalOutput")
    gout_d = nc.dram_tensor("gout", [G, GPC], F32, kind="ExternalOutput")

    with TileContext(nc) as tc:
        with (
            tc.tile_pool(name="const", bufs=1) as cst,
            tc.tile_pool(name="stream", bufs=5) as stp,
            tc.tile_pool(name="act", bufs=4) as actp,
            tc.tile_pool(name="xgp", bufs=4) as xgp,
            tc.tile_pool(name="psA", bufs=4, space=bass.MemorySpace.PSUM) as psA,
            tc.tile_pool(name="psB", bufs=2, space=bass.MemorySpace.PSUM) as psB,
            tc.tile_pool(name="psC", bufs=1, space=bass.MemorySpace.PSUM) as psC,
            tc.tile_pool(name="psD", bufs=1, space=bass.MemorySpace.PSUM) as psD,
        ):
            # --- constants (3 packed loads; issued after the first
            # compute-critical stream pieces, see the node loop) ---
            cAt = cst.tile([128, 256], F32)
            cBt = cst.tile([128, 128], BF16)
            indt_t = cst.tile([128, PT * GPC], F8)
            cGt = cst.tile([128, 934], F32)

            b1t = cAt[:, 0:128]          # col = 2n + mh
            b2t = cAt[:, 128:192]        # col = n
            b3t = cAt[:, 192:256]        # col = n (broadcast over partitions)
            indt = indt_t[:]             # col = t*16 + graph
            w3t = cBt[:]                 # col = 2n (odd cols zero)

            zeros = cst.tile([128, 256], F32)
            nc.gpsimd.memset(zeros[:], 0.0)

            ident = cst.tile([128, 128], F32)
            make_identity(nc, ident[:])

            def relu_bias(out, in_, bias, use_dve):
                """relu(in_ + bias): ACT or DVE (load balance)."""
                if not use_dve:
                    nc.scalar.activation(out, in_, RELU, bias=bias)
                else:
                    nc.vector.scalar_tensor_tensor(
                        out, in_, bias, zeros[:, :in_.shape[-1]],
                        mybir.AluOpType.add, mybir.AluOpType.max,
                    )

            # node_out staging in SBUF (written 4 nodes at a time)
            nout_sb = cst.tile([128, NPC], F32)
            # pooling accumulator (interleaved with the node loop)
            pp = psD.tile([GPC, 256], F32)

            xg_tiles = {}

            def pool_dma(q):
                xgt = xgp.tile([128, 1024], F8, tag="xg")
                nc.sync.dma_start(xgt[:], xg_d[q])
                xg_tiles[q] = xgt

            def pool_mms(q):
                xgt = xg_tiles.pop(q)
                for t8 in range(8):
                    t = 8 * q + t8
                    if t >= PT:
                        break
                    nc.tensor.matmul(
                        pp[:],
                        indt[:, t * GPC:(t + 1) * GPC],
                        xgt[:, t8 * 256:(t8 + 1) * 256],
                        start=(t == 0), stop=(t == PT - 1),
                        skip_group_check=True,
                    )

            # --- graph head steps, interleaved into late node groups ---
            gh = {}

            def graph_step(step):
                if step == 0:
                    # relu(mean) then transpose [GPC, 256] -> [256, GPC]
                    xgr = actp.tile([GPC, 256], F32, tag="xgr")
                    nc.scalar.activation(xgr[:], pp[:], RELU)
                    gh["xgr"] = xgr
                elif step == 1:
                    xgt2 = actp.tile([128, 2 * GPC], F32, tag="xgt")
                    for kh in range(2):
                        ptr = psC.tile([128, GPC], F32, tag="p3")
                        nc.tensor.transpose(
                            ptr[:], gh["xgr"][:, kh * 128:(kh + 1) * 128],
                            ident[:GPC, :GPC],
                        )
                        nc.vector.tensor_copy(
                            xgt2[:, kh * GPC:(kh + 1) * GPC], ptr[:]
                        )
                    gh["xgt2"] = xgt2
                elif step == 2:
                    # layer 1: relu(x_graph) @ gs_w1 + gs_b1  (no relu after)
                    g1 = psC.tile([128, GPC], F32, tag="p3")
                    for kh in range(2):
                        nc.tensor.matmul(
                            g1[:], cGt[:, kh * 128:(kh + 1) * 128],
                            gh["xgt2"][:, kh * GPC:(kh + 1) * GPC],
                            start=(kh == 0), stop=(kh == 1),
                        )
                    a1 = actp.tile([128, GPC], F32, tag="ga")
                    nc.scalar.activation(a1[:], g1[:], IDENT,
                                         bias=cGt[:, 928:929])
                    gh["a1"] = a1
                elif step == 3:
                    # layer 2: relu(a1 @ gs_w2 + gs_b2)
                    g2 = psC.tile([128, GPC], F32, tag="p3")
                    nc.tensor.matmul(g2[:], cGt[:, 256:384], gh["a1"][:],
                                     start=True, stop=True)
                    a2 = actp.tile([128, GPC], F32, tag="ga")
                    nc.scalar.activation(a2[:], g2[:], RELU,
                                         bias=cGt[:, 929:930])
                    gh["a2"] = a2
                elif step == 4:
                    # layer 3: relu(a2 @ gh_w1 + gh_b1)  (D1=256 -> two halves)
                    a3 = actp.tile([128, 2 * GPC], F32, tag="ga3")
                    for mh in range(2):
                        g3 = psC.tile([128, GPC], F32, tag="p3")
                        nc.tensor.matmul(
                            g3[:], cGt[:, 384 + mh * 128:384 + (mh + 1) * 128],
                            gh["a2"][:], start=True, stop=True,
                        )
                        nc.scalar.activation(
                            a3[:, mh * GPC:(mh + 1) * GPC], g3[:], RELU,
                            bias=cGt[:, 930 + mh:931 + mh],
                        )
                    gh["a3"] = a3
                elif step == 5:
                    # layer 4: relu(a3 @ gh_w2 + gh_b2)
                    g4 = psC.tile([128, GPC], F32, tag="p3")
                    for kh in range(2):
                        nc.tensor.matmul(
                            g4[:], cGt[:, 640 + kh * 128:640 + (kh + 1) * 128],
                            gh["a3"][:, kh * GPC:(kh + 1) * GPC],
                            start=(kh == 0), stop=(kh == 1),
                        )
                    a4 = actp.tile([128, GPC], F32, tag="ga")
                    nc.scalar.activation(a4[:], g4[:], RELU,
                                         bias=cGt[:, 932:933])
                    gh["a4"] = a4
                elif step == 6:
                    # layer 5: a4 @ gh_w3 + gh_b3
                    g5 = psC.tile([G, GPC], F32, tag="p3")
                    nc.tensor.matmul(g5[:], cGt[:, 896:928], gh["a4"][:],
                                     start=True, stop=True)
                    gout_sb = actp.tile([G, GPC], F32, tag="gout")
                    nc.scalar.activation(gout_sb[:], g5[:], IDENT,
                                         bias=cGt[:G, 933:934])
                    nc.sync.dma_start(gout_d[:], gout_sb[:])

            # --- node loop: flat software pipeline over 64 nodes ---
            # Slot i emits: L1(i); side work (pool / graph-head matmuls,
            # which depend only on prefetched data and act as spacers);
            # L2(i-2); L3(i-4). The 2-slot lag gives each relu ~0.7us to
            # drain before its consumer, so the PE never stalls and stays
            # at its boosted p-state.
            aps = {}
            side = []          # queued pool-matmul spacer thunks
            gh_step = [0]      # next graph-head step to emit
            side_pushed = [0]  # pool loads whose matmuls are already queued

            def emit_l1(i):
                g, v = i // 4, i % 4
                ap, w1t = aps[g]
                xt = ap(0, 1024)
                w1o = v * 512
                p1 = psA.tile([128, 256], F32, tag="p1")
                for mh in range(2):
                    for kh in range(2):
                        nc.tensor.matmul(
                            p1[:, mh * 128:(mh + 1) * 128],
                            w1t[:, w1o + kh * 256 + mh * 128:
                                 w1o + kh * 256 + (mh + 1) * 128],
                            xt[:, v * 256 + kh * 128:v * 256 + (kh + 1) * 128],
                            start=(kh == 0), stop=(kh == 1),
                        )
                h1 = actp.tile([128, 256], BF16, tag="h1", bufs=6)
                for mh in range(2):
                    relu_bias(
                        h1[:, mh * 128:(mh + 1) * 128],
                        p1[:, mh * 128:(mh + 1) * 128],
                        b1t[:, 2 * i + mh:2 * i + mh + 1],
                        use_dve=(mh == 1),
                    )
                return h1

            def emit_l2(i, h1):
                g, v = i // 4, i % 4
                ap, _ = aps[g]
                p2 = psB.tile([128, 128], F32, tag="p2")
                for dh in range(2):
                    nc.tensor.matmul(
                        p2[:],
                        ap(1024 + v * 256 + dh * 128,
                           1024 + v * 256 + (dh + 1) * 128),
                        h1[:, dh * 128:(dh + 1) * 128],
                        start=(dh == 0), stop=(dh == 1),
                    )
                h2 = actp.tile([128, 128], BF16, tag="h2", bufs=6)
                relu_bias(h2, p2[:], b2t[:, i:i + 1], use_dve=(i % 2 == 1))
                return h2

            p3_state = {}

            def emit_l3(i, h2):
                v = i % 4
                if v == 0:
                    p3_state["t"] = psC.tile([128, 8], F32, tag="p3", name="p3")
                p3 = p3_state["t"]
                nc.tensor.matmul(
                    p3[:, 2 * v:2 * v + 2],
                    h2[:],
                    w3t[:, 2 * i:2 * i + 2],
                    start=True, stop=True,
                    skip_group_check=True,
                )
                if v == 3:
                    k = i - 3
                    nc.vector.tensor_add(
                        nout_sb[:, k:k + 4],
                        p3[:, 0:8:2],
                        b3t[:, k:k + 4],
                    )

            def pool_thunk(q, t4):
                t = 4 * q + t4

                def run():
                    xgt = xg_tiles[q]
                    nc.tensor.matmul(
                        pp[:],
                        indt[:, t * GPC:(t + 1) * GPC],
                        xgt[:, t4 * 256:(t4 + 1) * 256],
                        start=(t == 0), stop=(t == PT - 1),
                        skip_group_check=True,
                    )
                return run

            h1s, h2s = {}, {}
            for i in range(NPC):
                g, v = i // 4, i % 4
                if v == 0:
                    if g < 2:
                        pt0 = stp.tile([128, 1024], BF16, tag="stP",
                                       bufs=4, name=f"stp{g}0")
                        nc.sync.dma_start(pt0[:], st_d[g, :, 0:1024])
                        w1t = stp.tile([128, 2048], F8E3, tag="w1",
                                       name=f"w1t{g}")
                        nc.sync.dma_start(w1t[:], w1_d[g])
                        pt1 = stp.tile([128, 1024], BF16, tag="stP",
                                       bufs=4, name=f"stp{g}1")
                        nc.sync.dma_start(pt1[:], st_d[g, :, 1024:2048])
                        if g == 0:
                            nc.sync.dma_start(cAt[:], cA_d[:])
                            nc.sync.dma_start(cBt[:], cB_d[:])
                            nc.sync.dma_start(indt_t[:], ind_d[:])

                        def ap(c0, c1, ps=(pt0, pt1)):
                            return ps[c0 // 1024][:, c0 % 1024:
                                                  c0 % 1024 + (c1 - c0)]
                    else:
                        st = stp.tile([128, 2048], BF16, tag="st")
                        nc.sync.dma_start(st[:], st_d[g])
                        w1t = stp.tile([128, 2048], F8E3, tag="w1",
                                       name=f"w1t{g}")
                        nc.sync.dma_start(w1t[:], w1_d[g])

                        def ap(c0, c1, s=st):
                            return s[:, c0:c1]
                    aps[g] = (ap, w1t)
                    if g == 1:
                        nc.sync.dma_start(cGt[:], cG_d[:])
                    # pool loads spread across the stream: 2/group for g<4,
                    # then 1/group until all 17 are out
                    nq = len(xg_tiles)
                    want = min(PQ, 2 * g + 2)
                    for q in range(nq, want):
                        pool_dma(q)
                    # push pool matmuls one group after their DMA so the
                    # in-order PE never waits on an in-flight transfer
                    if g >= 1:
                        prev = min(PQ, 2 * g)
                        for q in range(side_pushed[0], prev):
                            for t4 in range(4):
                                if 4 * q + t4 < PT:
                                    side.append(pool_thunk(q, t4))
                            side_pushed[0] = q + 1

                h1s[i] = emit_l1(i)
                # spacers: up to 2 queued pool ops per slot
                for _ in range(2):
                    if side:
                        side.pop(0)()
                # graph head: one step per half-group once pooling is done
                # (serially-dependent steps spaced ~2 slots apart)
                if not side and i >= 46 and v in (0, 2) and gh_step[0] < 7:
                    graph_step(gh_step[0])
                    gh_step[0] += 1
                if i - 2 >= 0:
                    h2s[i - 2] = emit_l2(i - 2, h1s.pop(i - 2))
                if i - 4 >= 0:
                    emit_l3(i - 4, h2s.pop(i - 4))

            # drain the pipeline
            for s in side:
                s()
            while gh_step[0] < 7:
                graph_step(gh_step[0])
                gh_step[0] += 1
            for i in (NPC - 2, NPC - 1):
                h2s[i] = emit_l2(i, h1s.pop(i))
            for i in range(NPC - 4, NPC):
                emit_l3(i, h2s.pop(i))

            nc.sync.dma_start(nout_d[:], nout_sb[:])

    nc.compile()
    return nc


def _prep_core_inputs(c, xbf, x8, batch, lo_hi, inv_counts, w1f8, nh_b1,
                      w2bf, nh_b2, w3bf, nh_b3, cG):
    ns = slice(c * NPC, (c + 1) * NPC)
    xv = xbf.reshape(B, N, H)

    # xt: [g, p(h%128), v, kh, b] -> [NG, 128, 1024]
    xt = (
        xv[:, ns, :]                              # [b, n, h]
        .reshape(B, NG, 4, 2, 128)                # b, g, v, kh, p
        .transpose(1, 4, 2, 3, 0)                 # g, p, v, kh, b
        .reshape(NG, 128, 1024)
    )

    # w1: per group 2048 cols; col = n2*512 + kh*256 + mh*128 + m
    w1 = np.ascontiguousarray(
        w1f8[ns]                                  # [n, h, d1]
        .reshape(NG, 2, 2, 2, 128, 2, 128)        # g, n2, j, kh, p, mh, m
        .transpose(0, 4, 1, 2, 3, 5, 6)           # g, p, n2, j, kh, mh, m
        .reshape(NG, 128, 2048)
    )

    # w2: per group 1024 cols; col = v*256 + dh*128 + e
    w2 = (
        w2bf[ns]                                  # [n, d1, d2]
        .reshape(NG, 4, 2, 128, 128)              # g, v, dh, p, e
        .transpose(0, 3, 1, 2, 4)                 # g, p, v, dh, e
        .reshape(NG, 128, 1024)
    )
    stream = np.ascontiguousarray(np.concatenate([xt, w2], axis=2))

    # w3 padded: col 2n = w3_n, odd cols zero
    w3 = np.zeros((128, 2 * NPC), NPBF)
    w3[:, 0::2] = w3bf[ns, :, 0].T

    cA = np.empty((128, 256), np.float32)
    cA[:, 0:128] = (
        nh_b1[ns].reshape(NPC, 2, 128).transpose(2, 0, 1).reshape(128, 2 * NPC)
    )
    cA[:, 128:192] = nh_b2[ns].T
    cA[:, 192:256] = np.broadcast_to(nh_b3[ns].reshape(1, NPC), (128, NPC))

    # pooling rows for graphs [GPC*c, GPC*(c+1)), 8 row-tiles per DMA load
    lo, hi = lo_hi[c]
    nrows = hi - lo
    xg = np.zeros((PQ * 4 * 128, 256), NPF8)
    xg[:nrows] = x8[lo:hi]
    xg = np.ascontiguousarray(
        xg.reshape(PQ, 4, 128, 256).transpose(0, 2, 1, 3).reshape(PQ, 128, 1024)
    )
    ind = np.zeros((PT * 128, GPC), np.float32)
    gl = batch[lo:hi] - GPC * c
    ind[np.arange(nrows), gl] = inv_counts[batch[lo:hi]]
    ind = (
        ind.reshape(PT, 128, GPC).transpose(1, 0, 2).reshape(128, PT * GPC)
    ).astype(NPF8)

    return {"st": stream, "w1": w1, "xg": xg, "cA": cA, "cB": w3,
            "ind": ind, "cG": cG}


def kernel(x, batch, gs_w1, gs_b1, gs_w2, gs_b2,
           gh_w1, gh_b1, gh_w2, gh_b2, gh_w3, gh_b3,
           nh_w1, nh_b1, nh_w2, nh_b2, nh_w3, nh_b3):
    x = np.asarray(x, np.float32)
    batch = np.asarray(batch, np.int32)

    counts = np.bincount(batch, minlength=B).astype(np.float32)
    inv_counts = np.where(counts > 0, 1.0 / np.maximum(counts, 1), 0.0).astype(
        np.float32
    )
    # row ranges per core (batch is sorted); must fit in the padded tile count
    bounds = np.searchsorted(batch, np.arange(0, B + 1, GPC))
    lo_hi = [(int(bounds[c]), int(bounds[c + 1])) for c in range(NCORES)]
    assert all(hi - lo <= PT * 128 for lo, hi in lo_hi), "graph slice too large"

    xbf = x.astype(NPBF)
    x8 = x.astype(NPF8)
    w1f8 = (np.asarray(nh_w1, np.float32) * 64.0).astype(NPF8E3)
    w2bf = (np.asarray(nh_w2, np.float32) / 64.0).astype(NPBF)
    w3bf = np.asarray(nh_w3, np.float32).astype(NPBF)
    nh_b1 = np.asarray(nh_b1, np.float32) * 64.0
    nh_b2 = np.asarray(nh_b2, np.float32)
    nh_b3 = np.asarray(nh_b3, np.float32)

    # graph-head consts: one packed f32 tensor (replicated on all cores)
    cG = np.zeros((128, 934), np.float32)
    cG[:, 0:256] = (
        np.asarray(gs_w1, np.float32).reshape(2, 128, 128)
        .transpose(1, 0, 2).reshape(128, 256)
    )
    cG[:, 256:384] = np.asarray(gs_w2, np.float32)
    cG[:, 384:640] = np.asarray(gh_w1, np.float32)
    cG[:, 640:896] = (
        np.asarray(gh_w2, np.float32).reshape(2, 128, 128)
        .transpose(1, 0, 2).reshape(128, 256)
    )
    cG[:, 896:928] = np.asarray(gh_w3, np.float32)
    cG[:, 928] = np.asarray(gs_b1, np.float32)
    cG[:, 929] = np.asarray(gs_b2, np.float32)
    cG[:, 930:932] = np.asarray(gh_b1, np.float32).reshape(2, 128).T
    cG[:, 932] = np.asarray(gh_b2, np.float32)
    cG[:G, 933] = np.asarray(gh_b3, np.float32)

    zero_bias = not (nh_b1.any() or nh_b2.any() or nh_b3.any())
    key = ("nc", zero_bias)
    if key not in _CACHE:
        _CACHE[key] = _build_nc(zero_bias)
    nc = _CACHE[key]

    in_maps = [
        _prep_core_inputs(c, xbf, x8, batch, lo_hi, inv_counts, w1f8, nh_b1,
                          w2bf, nh_b2, w3bf, nh_b3, cG)
        for c in range(NCORES)
    ]

    res = run_bass_kernel_spmd(nc, in_maps, core_ids=list(range(NCORES)))
    _CACHE["last_result"] = res

    out = np.empty((B, G + N), np.float32)
    for c in range(NCORES):
        out[GPC * c:GPC * (c + 1), :G] = res.results[c]["gout"].T
        out[:, G + NPC * c:G + NPC * (c + 1)] = res.results[c]["nout"]
    return out
